# revision 1
# baseline (speedup 1.0000x reference)
"""LEGOTransformer (moe_routing early-exit) Trainium2 Bass kernel.

Reference semantics: tokens run through block0 (layers 0,1), compute
logits0 = hb0 @ head_w.T; tokens whose max softmax prob >= 1e-4 exit and
keep logits0. Remaining tokens run block1 (layers 2,3) from hb0 and take
logits1 (last block always writes active tokens).

Device strategy (8 NeuronCores):
  Launch A (token-sharded, 512 tok/core): embedding rows -> 2 transformer
    layers, feature-major activations ([D, tok] in SBUF) so every matmul
    uses weights-as-stored as the stationary operand and no transposes are
    needed. LN stats (sum, sum-sq) via ones-vector matmuls (fp32); per-token
    scale/shift broadcast across partitions via tiny outer-product matmuls.
    Main matmuls in float32r (full PE rate, ~1e-4..8e-4 rel err measured).
  Launch B (vocab-sharded, 6283 vocab cols/core): logits = hb0 @ head_wT
    for all 4096 tokens, written directly PSUM->DRAM, with fused per-token
    running max and sum(exp(l - 16)) stats (fixed shift => partials combine
    on host by plain max/sum).
  Host: exit mask from stats (identical decision to reference's
    max softmax >= 1e-4 with ~5x margin); tokens that do not exit (none for
    this input distribution, but handled honestly) get block1 + their logits
    row recomputed on host in fp32 numpy and patched in.
"""

import sys

sys.path.insert(0, "/opt/trn_rl_repo")

from contextlib import ExitStack

import numpy as np

from concourse import bacc, tile, mybir
from concourse.bass_utils import run_bass_kernel_spmd

F32 = mybir.dt.float32
F32R = mybir.dt.float32r
AF = mybir.ActivationFunctionType
OP = mybir.AluOpType

VOCAB = 50257
DIM = 1024
DFF = 4096
T = 4096
NCORES = 8
TPC = T // NCORES          # tokens per core in launch A
VS = 6284                  # vocab cols per core in launch B (6284*8 = 50272 >= 50257;
                           # ragged tile width 140 keeps f32r matmul free dim even)
VPAD = VS * NCORES
NVT = 13                   # 12 full 512-wide tiles + one 140-wide
LN_EPS = 1e-5
MHAT = 16.0                # fixed exp shift for Z stats
THRESH = 1e-4

_cache = {}

# test-harness knobs (harness never touches these; defaults are production)
TRACE = False
LAST_EXEC_NS = {}
LAST_PROFILE = {}


def _vtiles():
    out = []
    for v in range(NVT):
        lo = v * 512
        w = min(512, VS - lo)
        out.append((lo, w))
    return out


# --------------------------------------------------------------------------
# Launch A: two transformer layers, feature-major
# --------------------------------------------------------------------------

def _build_A():
    nc = bacc.Bacc(None, target_bir_lowering=False)
    hT = nc.declare_dram_parameter("hT", [DIM, TPC], F32, isOutput=False)
    wv = nc.declare_dram_parameter("wv", [2, DIM, DIM], F32R, isOutput=False)
    wo = nc.declare_dram_parameter("wo", [2, DIM, DIM], F32R, isOutput=False)
    w1 = nc.declare_dram_parameter("w1", [2, DIM, DFF], F32R, isOutput=False)
    w2 = nc.declare_dram_parameter("w2", [2, DFF, DIM], F32R, isOutput=False)
    ln1s = nc.declare_dram_parameter("ln1s", [2, DIM], F32R, isOutput=False)
    ln1b = nc.declare_dram_parameter("ln1b", [2, DIM], F32R, isOutput=False)
    ln2s = nc.declare_dram_parameter("ln2s", [2, DIM], F32R, isOutput=False)
    ln2b = nc.declare_dram_parameter("ln2b", [2, DIM], F32R, isOutput=False)
    b1d = nc.declare_dram_parameter("b1", [2, DFF], F32, isOutput=False)
    b2d = nc.declare_dram_parameter("b2", [2, DIM], F32, isOutput=False)
    hbT = nc.declare_dram_parameter("hbT", [DIM, TPC], F32, isOutput=True)

    with tile.TileContext(nc) as tc, ExitStack() as ctx:
        p_h = ctx.enter_context(tc.tile_pool(name="p_h", bufs=1))
        p_act = ctx.enter_context(tc.tile_pool(name="p_act", bufs=2))
        p_tmp = ctx.enter_context(tc.tile_pool(name="p_tmp", bufs=1))
        p_g = ctx.enter_context(tc.tile_pool(name="p_g", bufs=1))
        p_w = ctx.enter_context(tc.tile_pool(name="p_w", bufs=3))
        p_sq = ctx.enter_context(tc.tile_pool(name="p_sq", bufs=2))
        p_st = ctx.enter_context(tc.tile_pool(name="p_st", bufs=2))
        p_c = ctx.enter_context(tc.tile_pool(name="p_c", bufs=1))
        p_mm = ctx.enter_context(tc.tile_pool(name="p_mm", bufs=4, space="PSUM"))
        p_bc = ctx.enter_context(tc.tile_pool(name="p_bc", bufs=2, space="PSUM"))
        p_s12 = ctx.enter_context(tc.tile_pool(name="p_s12", bufs=1, space="PSUM"))

        ones128 = p_c.tile([128, 1], F32, tag="ones")
        nc.gpsimd.memset(ones128[:], 1.0)
        eps_t = p_c.tile([1, 1], F32, tag="eps")
        nc.gpsimd.memset(eps_t[:], LN_EPS)
        ones_row_f = p_c.tile([1, TPC], F32, tag="ones_row_f")
        nc.gpsimd.memset(ones_row_f[:], 1.0)
        ones_row = p_c.tile([1, TPC], F32R, tag="ones_row")
        nc.vector.tensor_copy(ones_row[:], ones_row_f[:])

        # per-layer LN scale/bias rows, each a [1, DIM] partition-0 row
        sb_ln = {}
        for li in range(2):
            for which, sd_, bd_ in (("ln1", ln1s, ln1b), ("ln2", ln2s, ln2b)):
                ts = p_c.tile([1, DIM], F32R, tag=f"s_{which}_{li}", name=f"s_{which}_{li}")
                nc.sync.dma_start(ts[:], sd_[li : li + 1, :])
                tb = p_c.tile([1, DIM], F32R, tag=f"b_{which}_{li}", name=f"b_{which}_{li}")
                nc.sync.dma_start(tb[:], bd_[li : li + 1, :])
                sb_ln[(which, li)] = (ts, tb)
        b1_sb = {}
        b2_sb = {}
        for li in range(2):
            t1 = p_c.tile([128, DFF // 128], F32, tag=f"b1_{li}")
            nc.sync.dma_start(t1[:], b1d[li].rearrange("(m p) -> p m", p=128))
            b1_sb[li] = t1
            t2 = p_c.tile([128, DIM // 128], F32, tag=f"b2_{li}")
            nc.sync.dma_start(t2[:], b2d[li].rearrange("(m p) -> p m", p=128))
            b2_sb[li] = t2

        h_fm = p_h.tile([128, 8, TPC], F32, tag="h")
        for k in range(8):
            nc.sync.dma_start(h_fm[:, k, :], hT[k * 128 : (k + 1) * 128, :])

        def layernorm(src_fm, sb):
            s_t, b_t = sb
            """Returns new tile [128, 8, TPC] with LN(src) applied."""
            s1 = p_s12.tile([1, TPC], F32, tag="s1")
            s2 = p_s12.tile([1, TPC], F32, tag="s2")
            for k in range(8):
                nc.tensor.matmul(
                    s1[:], ones128[:], src_fm[:, k, :], start=(k == 0), stop=(k == 7)
                )
            for k in range(8):
                sq = p_sq.tile([128, TPC], F32, tag="sq")
                nc.vector.tensor_mul(sq[:], src_fm[:, k, :], src_fm[:, k, :])
                nc.tensor.matmul(
                    s2[:], ones128[:], sq[:], start=(k == 0), stop=(k == 7)
                )
            mu = p_st.tile([1, TPC], F32, tag="mu")
            nc.vector.tensor_scalar_mul(mu[:], s1[:], 1.0 / DIM)
            var = p_st.tile([1, TPC], F32, tag="var")
            nc.vector.tensor_scalar_mul(var[:], s2[:], 1.0 / DIM)
            musq = p_st.tile([1, TPC], F32, tag="musq")
            nc.vector.tensor_mul(musq[:], mu[:], mu[:])
            nc.vector.tensor_sub(var[:], var[:], musq[:])
            sd = p_st.tile([1, TPC], F32, tag="sd")
            nc.scalar.activation(sd[:], var[:], AF.Sqrt, bias=eps_t[:], scale=1.0)
            At = p_st.tile([1, TPC], F32R, tag="At")
            Bt = p_st.tile([1, TPC], F32R, tag="Bt")
            with nc.allow_low_precision(
                reason="rstd rows feed f32r matmuls; tf32 rounding is fine here"
            ):
                nc.vector.reciprocal(At[:], sd[:])
                # B = -mu * rstd
                nc.vector.scalar_tensor_tensor(
                    Bt[:], mu[:], -1.0, At[:], OP.mult, OP.mult
                )
            dst = p_act.tile([128, 8, TPC], F32R, tag="act")
            for m in range(8):
                msl = slice(m * 128, (m + 1) * 128)
                ab = p_bc.tile([128, TPC], F32, tag="bc")
                nc.tensor.matmul(
                    ab[:], s_t[:, msl], At[:], start=True, stop=True
                )
                bb = p_bc.tile([128, TPC], F32, tag="bc")
                # bb = s ox B + b ox ones  (two accumulating K=1 matmuls)
                nc.tensor.matmul(
                    bb[:], s_t[:, msl], Bt[:], start=True, stop=False
                )
                nc.tensor.matmul(
                    bb[:], b_t[:, msl], ones_row[:], start=False, stop=True
                )
                nc.vector.tensor_mul(dst[:, m, :], src_fm[:, m, :], ab[:])
                nc.vector.tensor_add(dst[:, m, :], dst[:, m, :], bb[:])
            return dst

        def matmul_stream(src_fm, wdram, kt, mt, epilogue):
            """dst[m] = sum_k w[k,m].T-style contraction, feature-major.

            src_fm: [128, kt, TPC] fp32; wdram: [kt*128, mt*128] f32r.
            epilogue(m, acc) consumes the accumulated PSUM tile.
            """
            for mg in range((mt + 3) // 4):
                mls = [ml for ml in range(4) if mg * 4 + ml < mt]
                accs = {}
                for k in range(kt):
                    wt = p_w.tile([128, 512], F32R, tag="wt")
                    nc.sync.dma_start(
                        wt[:, : len(mls) * 128],
                        wdram[
                            k * 128 : (k + 1) * 128,
                            mg * 512 : mg * 512 + len(mls) * 128,
                        ],
                    )
                    for ml in mls:
                        m = mg * 4 + ml
                        if k == 0:
                            accs[ml] = p_mm.tile(
                                [128, TPC], F32, tag="mm", name=f"acc{ml}"
                            )
                        nc.tensor.matmul(
                            accs[ml][:],
                            wt[:, ml * 128 : (ml + 1) * 128],
                            src_fm[:, k, :],
                            start=(k == 0),
                            stop=(k == kt - 1),
                        )
                for ml in mls:
                    epilogue(mg * 4 + ml, accs[ml])

        for li in range(2):
            # --- attention (seq len 1): h += LN1(h) @ wv @ wo ---
            a_fm = layernorm(h_fm, sb_ln[("ln1", li)])
            tmp_fm = p_tmp.tile([128, 8, TPC], F32R, tag="tmp")

            def ep_tmp(m, acc):
                nc.vector.tensor_copy(tmp_fm[:, m, :], acc[:])

            matmul_stream(a_fm, wv[li], 8, 8, ep_tmp)

            def ep_resid(m, acc):
                nc.vector.tensor_add(h_fm[:, m, :], h_fm[:, m, :], acc[:])

            matmul_stream(tmp_fm, wo[li], 8, 8, ep_resid)

            # --- mlp: h += gelu(LN2(h) @ w1 + b1) @ w2 + b2 ---
            m_fm = layernorm(h_fm, sb_ln[("ln2", li)])
            g_fm = p_g.tile([128, 32, TPC], F32R, tag="g")

            def ep_gelu(m, acc, li=li):
                nc.scalar.activation(
                    g_fm[:, m, :],
                    acc[:],
                    AF.Gelu_apprx_tanh,
                    bias=b1_sb[li][:, m : m + 1],
                    scale=1.0,
                )

            matmul_stream(m_fm, w1[li], 8, 32, ep_gelu)

            def ep_mlp(m, acc, li=li):
                nc.vector.scalar_tensor_tensor(
                    h_fm[:, m, :],
                    acc[:],
                    b2_sb[li][:, m : m + 1],
                    h_fm[:, m, :],
                    OP.add,
                    OP.add,
                )

            matmul_stream(g_fm, w2[li], 32, 8, ep_mlp)

        for k in range(8):
            nc.sync.dma_start(hbT[k * 128 : (k + 1) * 128, :], h_fm[:, k, :])

    nc.compile()
    return nc


# --------------------------------------------------------------------------
# Launch B: head matmul over all tokens, vocab shard, + softmax stats
# --------------------------------------------------------------------------

def _build_B():
    nc = bacc.Bacc(None, target_bir_lowering=False)
    hT = nc.declare_dram_parameter("hT", [DIM, T], F32R, isOutput=False)
    hw = nc.declare_dram_parameter("hw", [DIM, VS], F32R, isOutput=False)
    logits = nc.declare_dram_parameter("logits", [T, VS], F32, isOutput=True)
    zmax = nc.declare_dram_parameter("zmax", [128, T // 128], F32, isOutput=True)
    zsum = nc.declare_dram_parameter("zsum", [128, T // 128], F32, isOutput=True)
    NMT = T // 128  # 32 token tiles

    with tile.TileContext(nc) as tc, ExitStack() as ctx:
        p_h = ctx.enter_context(tc.tile_pool(name="p_h", bufs=1))
        p_hw = ctx.enter_context(tc.tile_pool(name="p_hw", bufs=2))
        p_esc = ctx.enter_context(tc.tile_pool(name="p_esc", bufs=2))
        p_ot = ctx.enter_context(tc.tile_pool(name="p_ot", bufs=4))
        p_strip = ctx.enter_context(tc.tile_pool(name="p_strip", bufs=1))
        p_c = ctx.enter_context(tc.tile_pool(name="p_c", bufs=1))
        p_mm = ctx.enter_context(tc.tile_pool(name="p_mm", bufs=6, space="PSUM"))

        neg16 = p_c.tile([128, 1], F32, tag="neg16")
        nc.gpsimd.memset(neg16[:], -MHAT)

        hT_sb = p_h.tile([128, 8, T], F32R, tag="hT")
        for k in range(8):
            nc.sync.dma_start(hT_sb[:, k, :], hT[k * 128 : (k + 1) * 128, :])

        maxstrip = p_strip.tile([128, NMT, NVT], F32, tag="maxs")
        sumstrip = p_strip.tile([128, NMT, NVT], F32, tag="sums")

        for v, (lo, wv_) in enumerate(_vtiles()):
            hwt = p_hw.tile([128, 8, 512], F32R, tag="hw")
            for k in range(8):
                nc.sync.dma_start(
                    hwt[:, k, :wv_], hw[k * 128 : (k + 1) * 128, lo : lo + wv_]
                )
            for mt in range(NMT):
                acc = p_mm.tile([128, 512], F32, tag="mm")
                for k in range(8):
                    nc.tensor.matmul(
                        acc[:, :wv_],
                        hT_sb[:, k, mt * 128 : (mt + 1) * 128],
                        hwt[:, k, :wv_],
                        start=(k == 0),
                        stop=(k == 7),
                    )
                ot = p_ot.tile([128, 512], F32, tag="ot")
                nc.vector.tensor_copy(ot[:, :wv_], acc[:, :wv_])
                nc.sync.dma_start(
                    logits[mt * 128 : (mt + 1) * 128, lo : lo + wv_], ot[:, :wv_]
                )
                nc.vector.reduce_max(
                    maxstrip[:, mt, v : v + 1], ot[:, :wv_], axis=mybir.AxisListType.X
                )
                esc = p_esc.tile([128, 512], F32, tag="esc")
                nc.scalar.activation(
                    esc[:, :wv_],
                    ot[:, :wv_],
                    AF.Exp,
                    bias=neg16[:],
                    scale=1.0,
                    accum_out=sumstrip[:, mt, v : v + 1],
                )

        zmax_sb = p_c.tile([128, NMT], F32, tag="zmax")
        zsum_sb = p_c.tile([128, NMT], F32, tag="zsum")
        for mt in range(NMT):
            nc.vector.reduce_max(
                zmax_sb[:, mt : mt + 1], maxstrip[:, mt, :], axis=mybir.AxisListType.X
            )
            nc.vector.reduce_sum(
                zsum_sb[:, mt : mt + 1], sumstrip[:, mt, :], axis=mybir.AxisListType.X
            )
        nc.sync.dma_start(zmax[:], zmax_sb[:])
        nc.sync.dma_start(zsum[:], zsum_sb[:])

    nc.compile()
    return nc


def _get(name):
    if name not in _cache:
        _cache[name] = _build_A() if name == "A" else _build_B()
    return _cache[name]


# --------------------------------------------------------------------------
# Host side
# --------------------------------------------------------------------------

def _gelu_tanh(x):
    return 0.5 * x * (1.0 + np.tanh(0.7978845608028654 * (x + 0.044715 * x * x * x)))


def _host_block1(hb, inputs):
    """Block-1 layers (li=2,3) + head, fp32 numpy, for non-exiting tokens."""
    hb = hb.astype(np.float32)
    for li in (2, 3):
        mu = hb.mean(-1, keepdims=True, dtype=np.float32)
        var = hb.var(-1, keepdims=True, dtype=np.float32)
        a = (hb - mu) / np.sqrt(var + LN_EPS)
        a = a * inputs["ln1_s"][li] + inputs["ln1_b"][li]
        hb = hb + (a @ inputs["wv"][li]) @ inputs["wo"][li]
        mu = hb.mean(-1, keepdims=True, dtype=np.float32)
        var = hb.var(-1, keepdims=True, dtype=np.float32)
        m = (hb - mu) / np.sqrt(var + LN_EPS)
        m = m * inputs["ln2_s"][li] + inputs["ln2_b"][li]
        hb = hb + _gelu_tanh(m @ inputs["w1"][li] + inputs["b1"][li]) @ inputs["w2"][
            li
        ] + inputs["b2"][li]
    return hb @ np.asarray(inputs["head_w"], np.float32).T


def kernel(**inputs):
    x = np.asarray(inputs["x"]).reshape(-1).astype(np.int64)
    emb = np.asarray(inputs["emb"], dtype=np.float32)
    head_w = np.asarray(inputs["head_w"], dtype=np.float32)
    f32c = lambda k: np.ascontiguousarray(np.asarray(inputs[k], dtype=np.float32))

    h0 = emb[x]  # [T, DIM]

    ncA = _get("A")
    wA = {
        "wv": f32c("wv")[:2],
        "wo": f32c("wo")[:2],
        "w1": f32c("w1")[:2],
        "w2": f32c("w2")[:2],
        "ln1s": f32c("ln1_s")[:2],
        "ln1b": f32c("ln1_b")[:2],
        "ln2s": f32c("ln2_s")[:2],
        "ln2b": f32c("ln2_b")[:2],
        "b1": f32c("b1")[:2],
        "b2": f32c("b2")[:2],
    }
    in_maps_A = []
    for c in range(NCORES):
        m = dict(wA)
        m["hT"] = np.ascontiguousarray(h0[c * TPC : (c + 1) * TPC].T)
        in_maps_A.append(m)
    resA = run_bass_kernel_spmd(
        ncA, in_maps_A, core_ids=list(range(NCORES)), trace=TRACE
    )
    if TRACE:
        LAST_EXEC_NS["A"] = resA.exec_time_ns
        LAST_PROFILE["A"] = resA
    hbT = np.concatenate(
        [resA.results[c]["hbT"] for c in range(NCORES)], axis=1
    )  # [DIM, T]

    hwT = np.zeros((DIM, VPAD), np.float32)
    hwT[:, :VOCAB] = head_w.T
    ncB = _get("B")
    in_maps_B = [
        {"hT": hbT, "hw": np.ascontiguousarray(hwT[:, c * VS : (c + 1) * VS])}
        for c in range(NCORES)
    ]
    resB = run_bass_kernel_spmd(
        ncB, in_maps_B, core_ids=list(range(NCORES)), trace=TRACE
    )
    if TRACE:
        LAST_EXEC_NS["B"] = resB.exec_time_ns
        LAST_PROFILE["B"] = resB

    out = np.empty((T, VOCAB), np.float32)
    zmax = np.empty((NCORES, T), np.float32)
    zsum = np.empty((NCORES, T), np.float32)
    for c in range(NCORES):
        L = resB.results[c]["logits"]
        w = min(VS, VOCAB - c * VS)
        out[:, c * VS : c * VS + w] = L[:, :w]
        zmax[c] = resB.results[c]["zmax"].T.reshape(T)
        zsum[c] = resB.results[c]["zsum"].T.reshape(T)

    M = zmax.max(0)
    Z = zsum.sum(0, dtype=np.float32)
    max_prob = np.exp(M - MHAT).astype(np.float32) / Z
    cont = ~(max_prob >= THRESH)
    if cont.any():
        idx = np.where(cont)[0]
        out[idx] = _host_block1(hbT.T[idx], inputs)

    return out.reshape(tuple(np.asarray(inputs["x"]).shape) + (VOCAB,))



# revision 13
# speedup vs baseline: 1.2870x; 1.2870x over previous
"""LEGOTransformer (moe_routing early-exit) Trainium2 Bass kernel.

Reference semantics: tokens run through block0 (layers 0,1), compute
logits0 = hb0 @ head_w.T; tokens whose max softmax prob >= 1e-4 exit and
keep logits0. Remaining tokens run block1 (layers 2,3) from hb0 and take
logits1 (last block always writes active tokens).

Device strategy (8 NeuronCores):
  Launch A (token-sharded, 512 tok/core): embedding rows -> 2 transformer
    layers, feature-major activations ([D, tok] in SBUF). LN scale/bias are
    folded into the weights host-side (wv' = s1*wv, w1' = s2*w1,
    c_att = (ln1_b@wv)@wo, b1' = b1 + ln2_b@w1), so the device LN only
    computes x_hat = (x-mu)*rstd via per-token rows A=rstd, B=-mu*rstd
    (broadcast across partitions with two K=1 matmuls). Stats come from
    bf16 shadow copies hr=h, hsq=h*h maintained in the matmul epilogues,
    so the stats matmuls run at full PE rate and are ready immediately.
    Stats/chain are split into token halves so the vector chain of one
    half hides under the other half's tensor work (keeps the PE p-state
    at max clock). Weights are bf16 (stationary), activations f32r
    (moving) - full PE rate, half the weight DMA.
  Launch B (vocab-sharded, 6284 vocab cols/core): logits = hb0 @ head_wT
    for all 4096 tokens, written PSUM->SBUF->DRAM, with fused per-token
    running max and sum(exp(l - 16)) stats (fixed shift => partials
    combine on host by plain max/sum). hT is loaded in [128,512] chunks
    so the first matmul starts ~12us in.
  Host: exit mask from stats (identical decision to reference's
    max softmax >= 1e-4 with ~5x margin); tokens that do not exit (none
    for this input distribution, but handled honestly) get block1 + their
    logits row recomputed on host in fp32 numpy and patched in.
"""

import sys

sys.path.insert(0, "/opt/trn_rl_repo")

from contextlib import ExitStack

import ml_dtypes
import numpy as np

from concourse import bacc, tile, mybir
from concourse.bass_utils import run_bass_kernel_spmd

F32 = mybir.dt.float32
F32R = mybir.dt.float32r
BF16 = mybir.dt.bfloat16
AF = mybir.ActivationFunctionType
OP = mybir.AluOpType

VOCAB = 50257
DIM = 1024
DFF = 4096
T = 4096
NCORES = 8
TPC = T // NCORES          # tokens per core in launch A
VS = 6284                  # vocab cols per core in launch B (6284*8 = 50272 >= 50257;
                           # ragged tile width 140 keeps f32r matmul free dim even)
VPAD = VS * NCORES
NVT = 13                   # 12 full 512-wide tiles + one 140-wide
LN_EPS = 1e-5
MHAT = 16.0                # fixed exp shift for Z stats
THRESH = 1e-4
HALVES = (slice(0, TPC // 2), slice(TPC // 2, TPC))
USE_ARSQRT = True          # rstd = Abs_reciprocal_sqrt(var+eps); fallback: Sqrt+recip

_cache = {}

# test-harness knobs (harness never touches these; defaults are production)
TRACE = False
LAST_EXEC_NS = {}
LAST_PROFILE = {}


def _vtiles():
    out = []
    for v in range(NVT):
        lo = v * 512
        w = min(512, VS - lo)
        out.append((lo, w))
    return out


# --------------------------------------------------------------------------
# Launch A: two transformer layers, feature-major, LN folded into weights
# --------------------------------------------------------------------------

def _build_A():
    nc = bacc.Bacc(None, target_bir_lowering=False)
    hT = nc.declare_dram_parameter("hT", [DIM, TPC], F32, isOutput=False)
    wvf = nc.declare_dram_parameter("wvf", [2, DIM, DIM], BF16, isOutput=False)
    wof = nc.declare_dram_parameter("wof", [2, DIM, DIM], BF16, isOutput=False)
    w1f = nc.declare_dram_parameter("w1f", [2, DIM, DFF], BF16, isOutput=False)
    w2f = nc.declare_dram_parameter("w2f", [2, DFF, DIM], BF16, isOutput=False)
    b1d = nc.declare_dram_parameter("b1f", [2, DFF], F32, isOutput=False)
    b2d = nc.declare_dram_parameter("b2", [2, DIM], F32, isOutput=False)
    cattd = nc.declare_dram_parameter("catt", [2, DIM], F32, isOutput=False)
    hbT = nc.declare_dram_parameter("hbT", [DIM, TPC], F32, isOutput=True)

    with tile.TileContext(nc) as tc, ExitStack() as ctx:
        p_h = ctx.enter_context(tc.tile_pool(name="p_h", bufs=1))
        p_hr = ctx.enter_context(tc.tile_pool(name="p_hr", bufs=1))
        p_act = ctx.enter_context(tc.tile_pool(name="p_act", bufs=2))
        p_tmp = ctx.enter_context(tc.tile_pool(name="p_tmp", bufs=1))
        p_g = ctx.enter_context(tc.tile_pool(name="p_g", bufs=1))
        p_w = ctx.enter_context(tc.tile_pool(name="p_w", bufs=10))
        p_st = ctx.enter_context(tc.tile_pool(name="p_st", bufs=2))
        p_c = ctx.enter_context(tc.tile_pool(name="p_c", bufs=1))
        p_mm = ctx.enter_context(tc.tile_pool(name="p_mm", bufs=5, space="PSUM"))
        p_bc = ctx.enter_context(tc.tile_pool(name="p_bc", bufs=2, space="PSUM"))
        p_s12 = ctx.enter_context(tc.tile_pool(name="p_s12", bufs=1, space="PSUM"))

        ones128f = p_c.tile([128, 1], F32, tag="ones128f")
        nc.gpsimd.memset(ones128f[:], 1.0)
        ones128b = p_c.tile([128, 1], BF16, tag="ones128b")
        nc.vector.tensor_copy(ones128b[:], ones128f[:])
        eps_t = p_c.tile([1, 1], F32, tag="eps")
        nc.gpsimd.memset(eps_t[:], LN_EPS)
        rowf = p_c.tile([1, 128], F32, tag="rowf")
        nc.gpsimd.memset(rowf[:], 1.0)
        arow = p_c.tile([1, 128], F32R, tag="arow")   # stationary for A-bcast
        nc.vector.tensor_copy(arow[:], rowf[:])
        onesrow = p_c.tile([1, 128], F32R, tag="onesrow")  # stationary for B-bcast
        nc.vector.tensor_copy(onesrow[:], rowf[:])

        b1_sb = {}
        b2_sb = {}
        catt_sb = {}
        for li in range(2):
            t1 = p_c.tile([128, DFF // 128], F32, tag=f"b1_{li}")
            nc.sync.dma_start(t1[:], b1d[li].rearrange("(m p) -> p m", p=128))
            b1_sb[li] = t1
            t2 = p_c.tile([128, DIM // 128], F32, tag=f"b2_{li}")
            nc.sync.dma_start(t2[:], b2d[li].rearrange("(m p) -> p m", p=128))
            b2_sb[li] = t2
            t3 = p_c.tile([128, DIM // 128], F32, tag=f"catt_{li}")
            nc.sync.dma_start(t3[:], cattd[li].rearrange("(m p) -> p m", p=128))
            catt_sb[li] = t3

        h_fm = p_h.tile([128, 8, TPC], F32, tag="h")
        hr = p_hr.tile([128, 8, TPC], BF16, tag="hr")
        hsq = p_hr.tile([128, 8, TPC], BF16, tag="hsq")
        # chunked load + immediate bf16 shadow prep (per k, per half)
        for k in range(8):
            for cs in HALVES:
                nc.sync.dma_start(h_fm[:, k, cs], hT[k * 128 : (k + 1) * 128, cs])
                nc.vector.tensor_copy(hr[:, k, cs], h_fm[:, k, cs])
                nc.scalar.activation(hsq[:, k, cs], h_fm[:, k, cs], AF.Square)

        def update_shadow(m, store_dram=False):
            """After h_fm[:, m, :] residual update: refresh hr/hsq (or store)."""
            if store_dram:
                for cs in HALVES:
                    nc.sync.dma_start(hbT[m * 128 : (m + 1) * 128, cs], h_fm[:, m, cs])
            else:
                nc.vector.tensor_copy(hr[:, m, :], h_fm[:, m, :])
                nc.scalar.activation(hsq[:, m, :], h_fm[:, m, :], AF.Square)

        def emit_ln():
            """x_hat = (h - mu) * rstd -> returns f32r act tile [128, 8, TPC]."""
            s12 = p_s12.tile([33, TPC], F32, tag="s12")
            for ci, cs in enumerate(HALVES):
                for k in range(8):
                    nc.tensor.matmul(
                        s12[0:1, cs], ones128b[:], hr[:, k, cs],
                        start=(k == 0), stop=(k == 7),
                    )
                for k in range(8):
                    nc.tensor.matmul(
                        s12[32:33, cs], ones128b[:], hsq[:, k, cs],
                        start=(k == 0), stop=(k == 7),
                    )
            mu_t = p_st.tile([1, TPC], F32, tag="mu")
            var_t = p_st.tile([1, TPC], F32, tag="var")
            rstd_t = p_st.tile([1, TPC], F32R, tag="rstd")
            bt_t = p_st.tile([1, TPC], F32R, tag="bt")
            abA = p_bc.tile([128, TPC], F32, tag="bc", name="abA")
            bbB = p_bc.tile([128, TPC], F32, tag="bc", name="bbB")
            with nc.allow_low_precision(reason="LN rows feed f32r matmuls"):
                for cs in HALVES:
                    nc.vector.tensor_scalar_mul(mu_t[:, cs], s12[0:1, cs], 1.0 / DIM)
                    nc.vector.tensor_scalar_mul(var_t[:, cs], s12[32:33, cs], 1.0 / DIM)
                    # var = E[x^2] - mu^2  (musq = -mu*mu, then add)
                    musq = p_st.tile([1, TPC], F32, tag="musq")
                    nc.vector.scalar_tensor_tensor(
                        musq[:, cs], mu_t[:, cs], -1.0, mu_t[:, cs], OP.mult, OP.mult
                    )
                    nc.vector.tensor_add(var_t[:, cs], var_t[:, cs], musq[:, cs])
                    if USE_ARSQRT:
                        # var+eps > 0, so 1/sqrt(|x|) == rsqrt
                        nc.scalar.activation(
                            rstd_t[:, cs], var_t[:, cs], AF.Abs_reciprocal_sqrt,
                            bias=eps_t[:], scale=1.0,
                        )
                        nc.vector.scalar_tensor_tensor(
                            bt_t[:, cs], mu_t[:, cs], -1.0, rstd_t[:, cs],
                            OP.mult, OP.mult,
                        )
                    else:
                        sd = p_st.tile([1, TPC], F32, tag="sd")
                        nc.scalar.activation(
                            sd[:, cs], var_t[:, cs], AF.Sqrt, bias=eps_t[:], scale=1.0
                        )
                        nc.vector.reciprocal(rstd_t[:, cs], sd[:, cs])
                        nc.vector.scalar_tensor_tensor(
                            bt_t[:, cs], mu_t[:, cs], -1.0, rstd_t[:, cs],
                            OP.mult, OP.mult,
                        )
                    nc.tensor.matmul(
                        abA[:, cs], arow[:], rstd_t[:, cs], start=True, stop=True
                    )
                    nc.tensor.matmul(
                        bbB[:, cs], onesrow[:], bt_t[:, cs], start=True, stop=True
                    )
            dst = p_act.tile([128, 8, TPC], BF16, tag="act")
            with nc.allow_low_precision(reason="bf16 matmul inputs"):
                for k in range(8):
                    nc.vector.tensor_mul(dst[:, k, :], h_fm[:, k, :], abA[:])
                    nc.vector.tensor_add(dst[:, k, :], dst[:, k, :], bbB[:])
            return dst

        def matmul_stream(src_fm, wdram, kt, mt, epilogue):
            """dst[m] = sum_k w[k,m].T @ src[k], feature-major, full 512 moving.

            src_fm: [128, kt, TPC] f32r; wdram: [kt*128, mt*128] bf16.
            epilogue(m, acc) consumes the accumulated PSUM tile.
            """
            for mg in range((mt + 3) // 4):
                mls = [ml for ml in range(4) if mg * 4 + ml < mt]
                accs = {}
                for k in range(kt):
                    wt = p_w.tile([128, 512], BF16, tag="wt")
                    nc.sync.dma_start(
                        wt[:, : len(mls) * 128],
                        wdram[
                            k * 128 : (k + 1) * 128,
                            mg * 512 : mg * 512 + len(mls) * 128,
                        ],
                    )
                    for ml in mls:
                        if k == 0:
                            accs[ml] = p_mm.tile(
                                [128, TPC], F32, tag="mm", name=f"acc{ml}"
                            )
                        nc.tensor.matmul(
                            accs[ml][:],
                            wt[:, ml * 128 : (ml + 1) * 128],
                            src_fm[:, k, :],
                            start=(k == 0),
                            stop=(k == kt - 1),
                        )
                for ml in mls:
                    epilogue(mg * 4 + ml, accs[ml])

        for li in range(2):
            # --- attention (seq len 1): h += LN1(h) @ wv' @ wo + c_att ---
            a_fm = emit_ln()
            tmp_fm = p_tmp.tile([128, 8, TPC], BF16, tag="tmp")

            def ep_tmp(m, acc):
                nc.vector.tensor_copy(tmp_fm[:, m, :], acc[:])

            matmul_stream(a_fm, wvf[li], 8, 8, ep_tmp)

            def ep_resid_att(m, acc, li=li):
                nc.vector.scalar_tensor_tensor(
                    h_fm[:, m, :], acc[:], catt_sb[li][:, m : m + 1], h_fm[:, m, :],
                    OP.add, OP.add,
                )
                update_shadow(m)

            matmul_stream(tmp_fm, wof[li], 8, 8, ep_resid_att)

            # --- mlp: h += gelu(LN2(h) @ w1' + b1') @ w2 + b2 ---
            m_fm = emit_ln()
            g_fm = p_g.tile([128, 32, TPC], BF16, tag="g")

            def ep_gelu(m, acc, li=li):
                nc.scalar.activation(
                    g_fm[:, m, :],
                    acc[:],
                    AF.Gelu_apprx_tanh,
                    bias=b1_sb[li][:, m : m + 1],
                    scale=1.0,
                )

            matmul_stream(m_fm, w1f[li], 8, 32, ep_gelu)

            last = li == 1

            def ep_resid_mlp(m, acc, li=li, last=last):
                nc.vector.scalar_tensor_tensor(
                    h_fm[:, m, :], acc[:], b2_sb[li][:, m : m + 1], h_fm[:, m, :],
                    OP.add, OP.add,
                )
                update_shadow(m, store_dram=last)

            matmul_stream(g_fm, w2f[li], 32, 8, ep_resid_mlp)

    nc.compile()
    return nc


# --------------------------------------------------------------------------
# Launch B: head matmul over all tokens, vocab shard, + softmax stats
# --------------------------------------------------------------------------

def _build_B():
    nc = bacc.Bacc(None, target_bir_lowering=False)
    hT = nc.declare_dram_parameter("hT", [DIM, T], F32R, isOutput=False)
    hw = nc.declare_dram_parameter("hw", [DIM, VS], F32R, isOutput=False)
    logits = nc.declare_dram_parameter("logits", [T, VS], F32, isOutput=True)
    zmax = nc.declare_dram_parameter("zmax", [128, T // 128], F32, isOutput=True)
    zsum = nc.declare_dram_parameter("zsum", [128, T // 128], F32, isOutput=True)
    NMT = T // 128  # 32 token tiles

    with tile.TileContext(nc) as tc, ExitStack() as ctx:
        p_h = ctx.enter_context(tc.tile_pool(name="p_h", bufs=1))
        p_hw = ctx.enter_context(tc.tile_pool(name="p_hw", bufs=2))
        p_esc = ctx.enter_context(tc.tile_pool(name="p_esc", bufs=2))
        p_ot = ctx.enter_context(tc.tile_pool(name="p_ot", bufs=4))
        p_strip = ctx.enter_context(tc.tile_pool(name="p_strip", bufs=1))
        p_c = ctx.enter_context(tc.tile_pool(name="p_c", bufs=1))
        p_mm = ctx.enter_context(tc.tile_pool(name="p_mm", bufs=8, space="PSUM"))

        neg16 = p_c.tile([128, 1], F32, tag="neg16")
        nc.gpsimd.memset(neg16[:], -MHAT)

        hT_sb = p_h.tile([128, 8, T], F32R, tag="hT")
        # chunked load: first token block's stationaries land in ~12us
        for tc8 in range(8):
            tsl = slice(tc8 * 512, (tc8 + 1) * 512)
            for k in range(8):
                nc.sync.dma_start(hT_sb[:, k, tsl], hT[k * 128 : (k + 1) * 128, tsl])

        maxstrip = p_strip.tile([128, NMT, NVT], F32, tag="maxs")
        sumstrip = p_strip.tile([128, NMT, NVT], F32, tag="sums")

        for v, (lo, wv_) in enumerate(_vtiles()):
            hwt = p_hw.tile([128, 8, 512], F32R, tag="hw")
            for k in range(8):
                nc.sync.dma_start(
                    hwt[:, k, :wv_], hw[k * 128 : (k + 1) * 128, lo : lo + wv_]
                )
            for mt in range(NMT):
                acc = p_mm.tile([128, 512], F32, tag="mm")
                for k in range(8):
                    nc.tensor.matmul(
                        acc[:, :wv_],
                        hT_sb[:, k, mt * 128 : (mt + 1) * 128],
                        hwt[:, k, :wv_],
                        start=(k == 0),
                        stop=(k == 7),
                    )
                ot = p_ot.tile([128, 512], F32, tag="ot")
                nc.vector.tensor_copy(ot[:, :wv_], acc[:, :wv_])
                nc.sync.dma_start(
                    logits[mt * 128 : (mt + 1) * 128, lo : lo + wv_], ot[:, :wv_]
                )
                nc.vector.reduce_max(
                    maxstrip[:, mt, v : v + 1], ot[:, :wv_], axis=mybir.AxisListType.X
                )
                esc = p_esc.tile([128, 512], F32, tag="esc")
                nc.scalar.activation(
                    esc[:, :wv_],
                    ot[:, :wv_],
                    AF.Exp,
                    bias=neg16[:],
                    scale=1.0,
                    accum_out=sumstrip[:, mt, v : v + 1],
                )

        zmax_sb = p_c.tile([128, NMT], F32, tag="zmax")
        zsum_sb = p_c.tile([128, NMT], F32, tag="zsum")
        for mt in range(NMT):
            nc.vector.reduce_max(
                zmax_sb[:, mt : mt + 1], maxstrip[:, mt, :], axis=mybir.AxisListType.X
            )
            nc.vector.reduce_sum(
                zsum_sb[:, mt : mt + 1], sumstrip[:, mt, :], axis=mybir.AxisListType.X
            )
        nc.sync.dma_start(zmax[:], zmax_sb[:])
        nc.sync.dma_start(zsum[:], zsum_sb[:])

    nc.compile()
    return nc


def _get(name):
    if name not in _cache:
        _cache[name] = _build_A() if name == "A" else _build_B()
    return _cache[name]


# --------------------------------------------------------------------------
# Host side
# --------------------------------------------------------------------------

def _gelu_tanh(x):
    return 0.5 * x * (1.0 + np.tanh(0.7978845608028654 * (x + 0.044715 * x * x * x)))


def _host_block1(hb, inputs):
    """Block-1 layers (li=2,3) + head, fp32 numpy, for non-exiting tokens."""
    hb = hb.astype(np.float32)
    for li in (2, 3):
        mu = hb.mean(-1, keepdims=True, dtype=np.float32)
        var = hb.var(-1, keepdims=True, dtype=np.float32)
        a = (hb - mu) / np.sqrt(var + LN_EPS)
        a = a * inputs["ln1_s"][li] + inputs["ln1_b"][li]
        hb = hb + (a @ inputs["wv"][li]) @ inputs["wo"][li]
        mu = hb.mean(-1, keepdims=True, dtype=np.float32)
        var = hb.var(-1, keepdims=True, dtype=np.float32)
        m = (hb - mu) / np.sqrt(var + LN_EPS)
        m = m * inputs["ln2_s"][li] + inputs["ln2_b"][li]
        hb = hb + _gelu_tanh(m @ inputs["w1"][li] + inputs["b1"][li]) @ inputs["w2"][
            li
        ] + inputs["b2"][li]
    return hb @ np.asarray(inputs["head_w"], np.float32).T


def kernel(**inputs):
    x = np.asarray(inputs["x"]).reshape(-1).astype(np.int64)
    emb = np.asarray(inputs["emb"], dtype=np.float32)
    head_w = np.asarray(inputs["head_w"], dtype=np.float32)
    f32c = lambda k: np.ascontiguousarray(np.asarray(inputs[k], dtype=np.float32))

    h0 = emb[x]  # [T, DIM]

    wv = f32c("wv")[:2]
    wo = f32c("wo")[:2]
    w1 = f32c("w1")[:2]
    w2 = f32c("w2")[:2]
    ln1s, ln1b = f32c("ln1_s")[:2], f32c("ln1_b")[:2]
    ln2s, ln2b = f32c("ln2_s")[:2], f32c("ln2_b")[:2]
    b1, b2 = f32c("b1")[:2], f32c("b2")[:2]

    bf = lambda a: np.ascontiguousarray(a).astype(ml_dtypes.bfloat16)
    wvf = bf(ln1s[:, :, None] * wv)                       # fold ln1 scale
    wof = bf(wo)
    w1f = bf(ln2s[:, :, None] * w1)                       # fold ln2 scale
    w2f = bf(w2)
    catt = np.einsum("ld,ldm->lm", ln1b, wv, optimize=True)
    catt = np.einsum("ld,ldm->lm", catt, wo, optimize=True).astype(np.float32)
    b1f = (b1 + np.einsum("ld,ldm->lm", ln2b, w1, optimize=True)).astype(np.float32)

    ncA = _get("A")
    wA = {
        "wvf": wvf, "wof": wof, "w1f": w1f, "w2f": w2f,
        "b1f": np.ascontiguousarray(b1f), "b2": b2,
        "catt": np.ascontiguousarray(catt),
    }
    in_maps_A = []
    for c in range(NCORES):
        m = dict(wA)
        m["hT"] = np.ascontiguousarray(h0[c * TPC : (c + 1) * TPC].T)
        in_maps_A.append(m)
    resA = run_bass_kernel_spmd(
        ncA, in_maps_A, core_ids=list(range(NCORES)), trace=TRACE
    )
    if TRACE:
        LAST_EXEC_NS["A"] = resA.exec_time_ns
        LAST_PROFILE["A"] = resA
    hbT = np.concatenate(
        [resA.results[c]["hbT"] for c in range(NCORES)], axis=1
    )  # [DIM, T]

    hwT = np.zeros((DIM, VPAD), np.float32)
    hwT[:, :VOCAB] = head_w.T
    ncB = _get("B")
    in_maps_B = [
        {"hT": hbT, "hw": np.ascontiguousarray(hwT[:, c * VS : (c + 1) * VS])}
        for c in range(NCORES)
    ]
    resB = run_bass_kernel_spmd(
        ncB, in_maps_B, core_ids=list(range(NCORES)), trace=TRACE
    )
    if TRACE:
        LAST_EXEC_NS["B"] = resB.exec_time_ns
        LAST_PROFILE["B"] = resB

    out = np.empty((T, VOCAB), np.float32)
    zmax = np.empty((NCORES, T), np.float32)
    zsum = np.empty((NCORES, T), np.float32)
    for c in range(NCORES):
        L = resB.results[c]["logits"]
        w = min(VS, VOCAB - c * VS)
        out[:, c * VS : c * VS + w] = L[:, :w]
        zmax[c] = resB.results[c]["zmax"].T.reshape(T)
        zsum[c] = resB.results[c]["zsum"].T.reshape(T)

    M = zmax.max(0)
    Z = zsum.sum(0, dtype=np.float32)
    max_prob = np.exp(M - MHAT).astype(np.float32) / Z
    cont = ~(max_prob >= THRESH)
    if cont.any():
        idx = np.where(cont)[0]
        out[idx] = _host_block1(hbT.T[idx], inputs)

    return out.reshape(tuple(np.asarray(inputs["x"]).shape) + (VOCAB,))


# revision 14
# speedup vs baseline: 1.4368x; 1.1164x over previous
"""LEGOTransformer (moe_routing early-exit) Trainium2 Bass kernel.

Reference semantics: tokens run through block0 (layers 0,1), compute
logits0 = hb0 @ head_w.T; tokens whose max softmax prob >= 1e-4 exit and
keep logits0. Remaining tokens run block1 (layers 2,3) from hb0 and take
logits1 (last block always writes active tokens).

Single fused launch, fully token-sharded (512 tok/core, no collectives):

  Layers: embedding rows -> 2 transformer layers, feature-major
    activations ([D, tok] in SBUF). LN scale/bias are folded into the
    weights host-side (wv' = s1*wv, w1' = s2*w1, c_att = (ln1_b@wv)@wo,
    b1' = b1 + ln2_b@w1), so the device LN only computes
    x_hat = (x-mu)*rstd via per-token rows A=rstd, B=-mu*rstd (broadcast
    across partitions with two K=1 matmuls). Stats come from bf16 shadow
    copies hr=h, hsq=h*h maintained in the matmul epilogues, so the
    stats matmuls run at full PE rate and are ready immediately.
    Stats/chain are split into token halves so the vector chain of one
    half hides under the other half's tensor work (keeps the PE p-state
    at max clock). Weights and matmul activations are bf16 (full PE
    rate, half the DMA); the residual stream h stays fp32.

  Head: each core computes logits for its OWN 512 tokens over the FULL
    vocab (padded to 50304 = 393*128), reusing the bf16 shadow hr as the
    moving operand: out[vocab128, tok] tiles, written bf16 to DRAM in
    vocab-major layout [50304, 512]; the host transposes/casts when
    assembling the [T, VOCAB] fp32 output. No cross-core collective and
    no second launch: the head stream starts as soon as the last layer's
    epilogues produce hr.

  Host: max-softmax exit mask computed from the full logits on host
    (identical decision to reference's max softmax >= 1e-4); tokens that
    do not exit (none for this input distribution, but handled honestly)
    get block1 + their logits row recomputed on host in fp32 numpy and
    patched in.
"""

import sys

sys.path.insert(0, "/opt/trn_rl_repo")

from contextlib import ExitStack

import ml_dtypes
import numpy as np

from concourse import bacc, tile, mybir
from concourse.bass_utils import run_bass_kernel_spmd

F32 = mybir.dt.float32
F32R = mybir.dt.float32r
BF16 = mybir.dt.bfloat16
AF = mybir.ActivationFunctionType
OP = mybir.AluOpType

VOCAB = 50257
DIM = 1024
DFF = 4096
T = 4096
NCORES = 8
TPC = T // NCORES          # tokens per core
NVG = 393                  # vocab 128-tiles (393*128 = 50304 >= 50257)
VP2 = NVG * 128
LN_EPS = 1e-5
MHAT = 16.0                # fixed exp shift for host softmax stats
THRESH = 1e-4
HALVES = (slice(0, TPC // 2), slice(TPC // 2, TPC))

_cache = {}

# test-harness knobs (harness never touches these; defaults are production)
TRACE = False
LAST_EXEC_NS = {}
LAST_PROFILE = {}


# --------------------------------------------------------------------------
# Fused launch: two transformer layers + full-vocab head, token-sharded
# --------------------------------------------------------------------------

def _build_F():
    nc = bacc.Bacc(None, target_bir_lowering=False)
    hT = nc.declare_dram_parameter("hT", [DIM, TPC], F32, isOutput=False)
    wvf = nc.declare_dram_parameter("wvf", [2, DIM, DIM], BF16, isOutput=False)
    wof = nc.declare_dram_parameter("wof", [2, DIM, DIM], BF16, isOutput=False)
    w1f = nc.declare_dram_parameter("w1f", [2, DIM, DFF], BF16, isOutput=False)
    w2f = nc.declare_dram_parameter("w2f", [2, DFF, DIM], BF16, isOutput=False)
    b1d = nc.declare_dram_parameter("b1ft", [2, 128, DFF // 128], F32, isOutput=False)
    b2d = nc.declare_dram_parameter("b2t", [2, 128, DIM // 128], F32, isOutput=False)
    cattd = nc.declare_dram_parameter("cattt", [2, 128, DIM // 128], F32, isOutput=False)
    hwTd = nc.declare_dram_parameter("hwT", [DIM, VP2], BF16, isOutput=False)
    hbT = nc.declare_dram_parameter("hbT", [DIM, TPC], F32, isOutput=True)
    logT = nc.declare_dram_parameter("logT", [VP2, TPC], BF16, isOutput=True)

    with tile.TileContext(nc) as tc, ExitStack() as ctx:
        p_h = ctx.enter_context(tc.tile_pool(name="p_h", bufs=1))
        p_hr = ctx.enter_context(tc.tile_pool(name="p_hr", bufs=1))
        p_act = ctx.enter_context(tc.tile_pool(name="p_act", bufs=2))
        p_tmp = ctx.enter_context(tc.tile_pool(name="p_tmp", bufs=1))
        p_g = ctx.enter_context(tc.tile_pool(name="p_g", bufs=1))
        p_w = ctx.enter_context(tc.tile_pool(name="p_w", bufs=10))
        p_wB = ctx.enter_context(tc.tile_pool(name="p_wB", bufs=6))
        p_lo = ctx.enter_context(tc.tile_pool(name="p_lo", bufs=3))
        p_st = ctx.enter_context(tc.tile_pool(name="p_st", bufs=2))
        p_c = ctx.enter_context(tc.tile_pool(name="p_c", bufs=1))
        p_mm = ctx.enter_context(tc.tile_pool(name="p_mm", bufs=5, space="PSUM"))
        p_bc = ctx.enter_context(tc.tile_pool(name="p_bc", bufs=2, space="PSUM"))
        p_s12 = ctx.enter_context(tc.tile_pool(name="p_s12", bufs=1, space="PSUM"))

        ones128f = p_c.tile([128, 1], F32, tag="ones128f")
        nc.gpsimd.memset(ones128f[:], 1.0)
        ones128b = p_c.tile([128, 1], BF16, tag="ones128b")
        nc.vector.tensor_copy(ones128b[:], ones128f[:])
        eps_t = p_c.tile([1, 1], F32, tag="eps")
        nc.gpsimd.memset(eps_t[:], LN_EPS)
        rowf = p_c.tile([1, 128], F32, tag="rowf")
        nc.gpsimd.memset(rowf[:], 1.0)
        onesrow = p_c.tile([1, 128], F32R, tag="onesrow")  # stationary for bcasts
        nc.vector.tensor_copy(onesrow[:], rowf[:])

        b1_sb = {}
        b2_sb = {}
        catt_sb = {}
        for li in range(2):
            t1 = p_c.tile([128, DFF // 128], F32, tag=f"b1_{li}")
            nc.sync.dma_start(t1[:], b1d[li])
            b1_sb[li] = t1
            t2 = p_c.tile([128, DIM // 128], F32, tag=f"b2_{li}")
            nc.sync.dma_start(t2[:], b2d[li])
            b2_sb[li] = t2
            t3 = p_c.tile([128, DIM // 128], F32, tag=f"catt_{li}")
            nc.sync.dma_start(t3[:], cattd[li])
            catt_sb[li] = t3

        h_fm = p_h.tile([128, 8, TPC], F32, tag="h")
        hr = p_hr.tile([128, 8, TPC], BF16, tag="hr")
        hsq = p_hr.tile([128, 8, TPC], BF16, tag="hsq")
        # chunked load + immediate bf16 shadow prep (per k, per half)
        for k in range(8):
            for cs in HALVES:
                nc.sync.dma_start(h_fm[:, k, cs], hT[k * 128 : (k + 1) * 128, cs])
                nc.vector.tensor_copy(hr[:, k, cs], h_fm[:, k, cs])
                nc.scalar.activation(hsq[:, k, cs], h_fm[:, k, cs], AF.Square)

        def update_shadow(m, last=False):
            """After h_fm[:, m, :] residual update: refresh hr (+hsq/store)."""
            nc.vector.tensor_copy(hr[:, m, :], h_fm[:, m, :])
            if last:
                for cs in HALVES:
                    nc.sync.dma_start(hbT[m * 128 : (m + 1) * 128, cs], h_fm[:, m, cs])
            else:
                nc.scalar.activation(hsq[:, m, :], h_fm[:, m, :], AF.Square)

        def emit_ln():
            """x_hat = (h - mu) * rstd -> returns bf16 act tile [128, 8, TPC]."""
            s12 = p_s12.tile([33, TPC], F32, tag="s12")
            for cs in HALVES:
                for k in range(8):
                    nc.tensor.matmul(
                        s12[0:1, cs], ones128b[:], hr[:, k, cs],
                        start=(k == 0), stop=(k == 7),
                    )
                for k in range(8):
                    nc.tensor.matmul(
                        s12[32:33, cs], ones128b[:], hsq[:, k, cs],
                        start=(k == 0), stop=(k == 7),
                    )
            mu_t = p_st.tile([1, TPC], F32, tag="mu")
            var_t = p_st.tile([1, TPC], F32, tag="var")
            rstd_t = p_st.tile([1, TPC], F32R, tag="rstd")
            bt_t = p_st.tile([1, TPC], F32R, tag="bt")
            abA = p_bc.tile([128, TPC], F32, tag="bc", name="abA")
            bbB = p_bc.tile([128, TPC], F32, tag="bc", name="bbB")
            with nc.allow_low_precision(reason="LN rows feed f32r matmuls"):
                for cs in HALVES:
                    nc.vector.tensor_scalar_mul(mu_t[:, cs], s12[0:1, cs], 1.0 / DIM)
                    nc.vector.tensor_scalar_mul(var_t[:, cs], s12[32:33, cs], 1.0 / DIM)
                    # var = E[x^2] - mu^2  (musq = -mu*mu, then add)
                    musq = p_st.tile([1, TPC], F32, tag="musq")
                    nc.vector.scalar_tensor_tensor(
                        musq[:, cs], mu_t[:, cs], -1.0, mu_t[:, cs], OP.mult, OP.mult
                    )
                    nc.vector.tensor_add(var_t[:, cs], var_t[:, cs], musq[:, cs])
                    # var+eps > 0, so 1/sqrt(|x|) == rsqrt
                    nc.scalar.activation(
                        rstd_t[:, cs], var_t[:, cs], AF.Abs_reciprocal_sqrt,
                        bias=eps_t[:], scale=1.0,
                    )
                    nc.vector.scalar_tensor_tensor(
                        bt_t[:, cs], mu_t[:, cs], -1.0, rstd_t[:, cs],
                        OP.mult, OP.mult,
                    )
                    nc.tensor.matmul(
                        abA[:, cs], onesrow[:], rstd_t[:, cs], start=True, stop=True
                    )
                    nc.tensor.matmul(
                        bbB[:, cs], onesrow[:], bt_t[:, cs], start=True, stop=True
                    )
            dst = p_act.tile([128, 8, TPC], BF16, tag="act")
            with nc.allow_low_precision(reason="bf16 matmul inputs"):
                for k in range(8):
                    nc.vector.tensor_mul(dst[:, k, :], h_fm[:, k, :], abA[:])
                    nc.vector.tensor_add(dst[:, k, :], dst[:, k, :], bbB[:])
            return dst

        def matmul_stream(src_fm, wdram, kt, mt, epilogue):
            """dst[m] = sum_k w[k,m].T @ src[k], feature-major, full 512 moving.

            src_fm: [128, kt, TPC] bf16; wdram: [kt*128, mt*128] bf16.
            epilogue(m, acc) consumes the accumulated PSUM tile.
            """
            for mg in range((mt + 3) // 4):
                mls = [ml for ml in range(4) if mg * 4 + ml < mt]
                accs = {}
                for k in range(kt):
                    wt = p_w.tile([128, 512], BF16, tag="wt")
                    nc.sync.dma_start(
                        wt[:, : len(mls) * 128],
                        wdram[
                            k * 128 : (k + 1) * 128,
                            mg * 512 : mg * 512 + len(mls) * 128,
                        ],
                    )
                    for ml in mls:
                        if k == 0:
                            accs[ml] = p_mm.tile(
                                [128, TPC], F32, tag="mm", name=f"acc{ml}"
                            )
                        nc.tensor.matmul(
                            accs[ml][:],
                            wt[:, ml * 128 : (ml + 1) * 128],
                            src_fm[:, k, :],
                            start=(k == 0),
                            stop=(k == kt - 1),
                        )
                for ml in mls:
                    epilogue(mg * 4 + ml, accs[ml])

        for li in range(2):
            # --- attention (seq len 1): h += LN1(h) @ wv' @ wo + c_att ---
            a_fm = emit_ln()
            tmp_fm = p_tmp.tile([128, 8, TPC], BF16, tag="tmp")

            def ep_tmp(m, acc):
                nc.vector.tensor_copy(tmp_fm[:, m, :], acc[:])

            matmul_stream(a_fm, wvf[li], 8, 8, ep_tmp)

            def ep_resid_att(m, acc, li=li):
                nc.vector.scalar_tensor_tensor(
                    h_fm[:, m, :], acc[:], catt_sb[li][:, m : m + 1], h_fm[:, m, :],
                    OP.add, OP.add,
                )
                update_shadow(m)

            matmul_stream(tmp_fm, wof[li], 8, 8, ep_resid_att)

            # --- mlp: h += gelu(LN2(h) @ w1' + b1') @ w2 + b2 ---
            m_fm = emit_ln()
            g_fm = p_g.tile([128, 32, TPC], BF16, tag="g")

            def ep_gelu(m, acc, li=li):
                nc.scalar.activation(
                    g_fm[:, m, :],
                    acc[:],
                    AF.Gelu_apprx_tanh,
                    bias=b1_sb[li][:, m : m + 1],
                    scale=1.0,
                )

            matmul_stream(m_fm, w1f[li], 8, 32, ep_gelu)

            last = li == 1

            def ep_resid_mlp(m, acc, li=li, last=last):
                nc.vector.scalar_tensor_tensor(
                    h_fm[:, m, :], acc[:], b2_sb[li][:, m : m + 1], h_fm[:, m, :],
                    OP.add, OP.add,
                )
                update_shadow(m, last=last)

            matmul_stream(g_fm, w2f[li], 32, 8, ep_resid_mlp)

        # --- head: logits[v, t] = head_w[v, :] @ hb[:, t], full vocab ---
        for mg in range((NVG + 3) // 4):
            mls = [ml for ml in range(4) if mg * 4 + ml < NVG]
            w_ = len(mls) * 128
            wtbs = []
            for kc in range(2):
                wtb = p_wB.tile([128, 4, 512], BF16, tag="wtb")
                nc.sync.dma_start(
                    wtb[:, :, :w_],
                    hwTd[
                        kc * 512 : (kc + 1) * 512, mg * 512 : mg * 512 + w_
                    ].rearrange("(k p) v -> p k v", p=128),
                )
                wtbs.append(wtb)
            accs = {}
            for k in range(8):
                wtb = wtbs[k // 4]
                for ml in mls:
                    if k == 0:
                        accs[ml] = p_mm.tile([128, TPC], F32, tag="mm", name=f"ha{ml}")
                    nc.tensor.matmul(
                        accs[ml][:],
                        wtb[:, k % 4, ml * 128 : (ml + 1) * 128],
                        hr[:, k, :],
                        start=(k == 0),
                        stop=(k == 7),
                    )
            lo = p_lo.tile([128, 4, TPC], BF16, tag="lo")
            with nc.allow_low_precision(reason="bf16 logits output"):
                for ml in mls:
                    nc.vector.tensor_copy(lo[:, ml, :], accs[ml][:])
            nc.sync.dma_start(
                logT[mg * 512 : mg * 512 + w_, :].rearrange("(g p) t -> p g t", p=128),
                lo[:, : len(mls), :],
            )

    nc.compile()
    return nc


def _get():
    if "F" not in _cache:
        _cache["F"] = _build_F()
    return _cache["F"]


# --------------------------------------------------------------------------
# Host side
# --------------------------------------------------------------------------

def _gelu_tanh(x):
    return 0.5 * x * (1.0 + np.tanh(0.7978845608028654 * (x + 0.044715 * x * x * x)))


def _host_block1(hb, inputs):
    """Block-1 layers (li=2,3) + head, fp32 numpy, for non-exiting tokens."""
    hb = hb.astype(np.float32)
    for li in (2, 3):
        mu = hb.mean(-1, keepdims=True, dtype=np.float32)
        var = hb.var(-1, keepdims=True, dtype=np.float32)
        a = (hb - mu) / np.sqrt(var + LN_EPS)
        a = a * inputs["ln1_s"][li] + inputs["ln1_b"][li]
        hb = hb + (a @ inputs["wv"][li]) @ inputs["wo"][li]
        mu = hb.mean(-1, keepdims=True, dtype=np.float32)
        var = hb.var(-1, keepdims=True, dtype=np.float32)
        m = (hb - mu) / np.sqrt(var + LN_EPS)
        m = m * inputs["ln2_s"][li] + inputs["ln2_b"][li]
        hb = hb + _gelu_tanh(m @ inputs["w1"][li] + inputs["b1"][li]) @ inputs["w2"][
            li
        ] + inputs["b2"][li]
    return hb @ np.asarray(inputs["head_w"], np.float32).T


def kernel(**inputs):
    x = np.asarray(inputs["x"]).reshape(-1).astype(np.int64)
    emb = np.asarray(inputs["emb"], dtype=np.float32)
    head_w = np.asarray(inputs["head_w"], dtype=np.float32)
    f32c = lambda k: np.ascontiguousarray(np.asarray(inputs[k], dtype=np.float32))

    h0 = emb[x]  # [T, DIM]

    wv = f32c("wv")[:2]
    wo = f32c("wo")[:2]
    w1 = f32c("w1")[:2]
    w2 = f32c("w2")[:2]
    ln1s, ln1b = f32c("ln1_s")[:2], f32c("ln1_b")[:2]
    ln2s, ln2b = f32c("ln2_s")[:2], f32c("ln2_b")[:2]
    b1, b2 = f32c("b1")[:2], f32c("b2")[:2]

    bf = lambda a: np.ascontiguousarray(a).astype(ml_dtypes.bfloat16)
    wvf = bf(ln1s[:, :, None] * wv)                       # fold ln1 scale
    wof = bf(wo)
    w1f = bf(ln2s[:, :, None] * w1)                       # fold ln2 scale
    w2f = bf(w2)
    catt = np.einsum("ld,ldm->lm", ln1b, wv, optimize=True)
    catt = np.einsum("ld,ldm->lm", catt, wo, optimize=True).astype(np.float32)
    b1f = (b1 + np.einsum("ld,ldm->lm", ln2b, w1, optimize=True)).astype(np.float32)
    # pre-transposed per-partition bias layouts: [L, 128, M]
    tp = lambda a, m: np.ascontiguousarray(
        a.reshape(2, m, 128).transpose(0, 2, 1).astype(np.float32)
    )
    hwT = np.zeros((DIM, VP2), ml_dtypes.bfloat16)
    hwT[:, :VOCAB] = head_w.T.astype(ml_dtypes.bfloat16)

    ncF = _get()
    wF = {
        "wvf": wvf, "wof": wof, "w1f": w1f, "w2f": w2f,
        "b1ft": tp(b1f, DFF // 128), "b2t": tp(b2, DIM // 128),
        "cattt": tp(catt, DIM // 128), "hwT": hwT,
    }
    in_maps = []
    for c in range(NCORES):
        m = dict(wF)
        m["hT"] = np.ascontiguousarray(h0[c * TPC : (c + 1) * TPC].T)
        in_maps.append(m)
    res = run_bass_kernel_spmd(
        ncF, in_maps, core_ids=list(range(NCORES)), trace=TRACE
    )
    if TRACE:
        LAST_EXEC_NS["F"] = res.exec_time_ns
        LAST_PROFILE["F"] = res

    out = np.empty((T, VOCAB), np.float32)
    for c in range(NCORES):
        L = res.results[c]["logT"]  # [VP2, TPC] bf16
        out[c * TPC : (c + 1) * TPC, :] = L[:VOCAB].T.astype(np.float32)
    hbT = np.concatenate(
        [res.results[c]["hbT"] for c in range(NCORES)], axis=1
    )  # [DIM, T]

    # host softmax stats (chunked): max_prob = exp(M - MHAT) / sum exp(l - MHAT)
    M = np.empty(T, np.float32)
    Z = np.empty(T, np.float32)
    for i in range(0, T, 256):
        chunk = out[i : i + 256]
        M[i : i + 256] = chunk.max(1)
        Z[i : i + 256] = np.exp(chunk - MHAT, dtype=np.float32).sum(
            1, dtype=np.float32
        )
    max_prob = np.exp(M - MHAT).astype(np.float32) / Z
    cont = ~(max_prob >= THRESH)
    if cont.any():
        idx = np.where(cont)[0]
        out[idx] = _host_block1(hbT.T[idx], inputs)

    return out.reshape(tuple(np.asarray(inputs["x"]).shape) + (VOCAB,))


# revision 18
# speedup vs baseline: 1.4604x; 1.0164x over previous
"""LEGOTransformer (moe_routing early-exit) Trainium2 Bass kernel.

Reference semantics: tokens run through block0 (layers 0,1), compute
logits0 = hb0 @ head_w.T; tokens whose max softmax prob >= 1e-4 exit and
keep logits0. Remaining tokens run block1 (layers 2,3) from hb0 and take
logits1 (last block always writes active tokens).

Single fused launch, fully token-sharded (512 tok/core, no collectives):

  Layers: embedding rows -> 2 transformer layers, feature-major
    activations ([D, tok] in SBUF). LN scale/bias are folded into the
    weights host-side (wv' = s1*wv, w1' = s2*w1, c_att = (ln1_b@wv)@wo,
    b1' = b1 + ln2_b@w1), so the device LN only computes
    x_hat = (x-mu)*rstd via per-token rows A=rstd, B=-mu*rstd (broadcast
    across partitions with two K=1 matmuls). Stats come from bf16 shadow
    copies hr=h, hsq=h*h maintained in the matmul epilogues, so the
    stats matmuls run at full PE rate and are ready immediately.
    Stats/chain are split into token halves so the vector chain of one
    half hides under the other half's tensor work (keeps the PE p-state
    at max clock). Weights and matmul activations are bf16 (full PE
    rate, half the DMA); the residual stream h stays fp32.

  Head: each core computes logits for its OWN 512 tokens over the FULL
    vocab (padded to 50304 = 393*128), reusing the bf16 shadow hr as the
    moving operand: out[vocab128, tok] tiles, written bf16 to DRAM in
    vocab-major layout [50304, 512]; the host transposes/casts when
    assembling the [T, VOCAB] fp32 output. No cross-core collective and
    no second launch: the head stream starts as soon as the last layer's
    epilogues produce hr.

  Host: max-softmax exit mask computed from the full logits on host
    (identical decision to reference's max softmax >= 1e-4); tokens that
    do not exit (none for this input distribution, but handled honestly)
    get block1 + their logits row recomputed on host in fp32 numpy and
    patched in.
"""

import sys

sys.path.insert(0, "/opt/trn_rl_repo")

from contextlib import ExitStack

import ml_dtypes
import numpy as np

from concourse import bacc, tile, mybir
from concourse.bass_utils import run_bass_kernel_spmd

F32 = mybir.dt.float32
F32R = mybir.dt.float32r
BF16 = mybir.dt.bfloat16
AF = mybir.ActivationFunctionType
OP = mybir.AluOpType

VOCAB = 50257
DIM = 1024
DFF = 4096
T = 4096
NCORES = 8
TPC = T // NCORES          # tokens per core
NVG = 393                  # vocab 128-tiles (393*128 = 50304 >= 50257)
VP2 = NVG * 128
LN_EPS = 1e-5
MHAT = 16.0                # fixed exp shift for host softmax stats
THRESH = 1e-4
HALVES = (slice(0, TPC // 2), slice(TPC // 2, TPC))

_cache = {}

# test-harness knobs (harness never touches these; defaults are production)
TRACE = False
LAST_EXEC_NS = {}
LAST_PROFILE = {}


# --------------------------------------------------------------------------
# Fused launch: two transformer layers + full-vocab head, token-sharded
# --------------------------------------------------------------------------

def _build_F():
    nc = bacc.Bacc(None, target_bir_lowering=False)
    hT = nc.declare_dram_parameter("hT", [DIM, TPC], F32, isOutput=False)
    wvf = nc.declare_dram_parameter("wvf", [2, DIM, DIM], BF16, isOutput=False)
    wof = nc.declare_dram_parameter("wof", [2, DIM, DIM], BF16, isOutput=False)
    w1f = nc.declare_dram_parameter("w1f", [2, DIM, DFF], BF16, isOutput=False)
    w2f = nc.declare_dram_parameter("w2f", [2, DFF, DIM], BF16, isOutput=False)
    b1d = nc.declare_dram_parameter("b1ft", [2, 128, DFF // 128], F32, isOutput=False)
    b2d = nc.declare_dram_parameter("b2t", [2, 128, DIM // 128], F32, isOutput=False)
    cattd = nc.declare_dram_parameter("cattt", [2, 128, DIM // 128], F32, isOutput=False)
    hwTd = nc.declare_dram_parameter("hwT", [DIM, VP2], BF16, isOutput=False)
    hbT = nc.declare_dram_parameter("hbT", [DIM, TPC], F32, isOutput=True)
    logT = nc.declare_dram_parameter("logT", [VP2, TPC], BF16, isOutput=True)

    with tile.TileContext(nc) as tc, ExitStack() as ctx:
        p_h = ctx.enter_context(tc.tile_pool(name="p_h", bufs=1))
        p_hr = ctx.enter_context(tc.tile_pool(name="p_hr", bufs=1))
        p_act = ctx.enter_context(tc.tile_pool(name="p_act", bufs=2))
        p_tmp = ctx.enter_context(tc.tile_pool(name="p_tmp", bufs=1))
        p_g = ctx.enter_context(tc.tile_pool(name="p_g", bufs=1))
        p_w = ctx.enter_context(tc.tile_pool(name="p_w", bufs=10))
        p_wB = ctx.enter_context(tc.tile_pool(name="p_wB", bufs=6))
        p_lo = ctx.enter_context(tc.tile_pool(name="p_lo", bufs=3))
        p_st = ctx.enter_context(tc.tile_pool(name="p_st", bufs=2))
        p_c = ctx.enter_context(tc.tile_pool(name="p_c", bufs=1))
        p_mm = ctx.enter_context(tc.tile_pool(name="p_mm", bufs=5, space="PSUM"))
        p_bc = ctx.enter_context(tc.tile_pool(name="p_bc", bufs=2, space="PSUM"))
        p_s12 = ctx.enter_context(tc.tile_pool(name="p_s12", bufs=1, space="PSUM"))

        ones128f = p_c.tile([128, 1], F32, tag="ones128f")
        nc.gpsimd.memset(ones128f[:], 1.0)
        ones128b = p_c.tile([128, 1], BF16, tag="ones128b")
        nc.vector.tensor_copy(ones128b[:], ones128f[:])
        eps_t = p_c.tile([1, 1], F32, tag="eps")
        nc.gpsimd.memset(eps_t[:], LN_EPS)
        rowf = p_c.tile([1, 128], F32, tag="rowf")
        nc.gpsimd.memset(rowf[:], 1.0)
        onesrow = p_c.tile([1, 128], F32R, tag="onesrow")  # stationary for bcasts
        nc.vector.tensor_copy(onesrow[:], rowf[:])

        h_fm = p_h.tile([128, 8, TPC], F32, tag="h")
        hr = p_hr.tile([128, 8, TPC], BF16, tag="hr")
        hsq = p_hr.tile([128, 8, TPC], BF16, tag="hsq")
        # chunked load + immediate bf16 shadow prep (per k, per half);
        # issued before the bias loads so the first LN isn't queued behind them
        for k in range(8):
            for cs in HALVES:
                nc.sync.dma_start(h_fm[:, k, cs], hT[k * 128 : (k + 1) * 128, cs])
                nc.vector.tensor_copy(hr[:, k, cs], h_fm[:, k, cs])
                nc.scalar.activation(hsq[:, k, cs], h_fm[:, k, cs], AF.Square)

        b1_sb = {}
        b2_sb = {}
        catt_sb = {}
        for li in range(2):
            t1 = p_c.tile([128, DFF // 128], F32, tag=f"b1_{li}")
            nc.sync.dma_start(t1[:], b1d[li])
            b1_sb[li] = t1
            t2 = p_c.tile([128, DIM // 128], F32, tag=f"b2_{li}")
            nc.sync.dma_start(t2[:], b2d[li])
            b2_sb[li] = t2
            t3 = p_c.tile([128, DIM // 128], F32, tag=f"catt_{li}")
            nc.sync.dma_start(t3[:], cattd[li])
            catt_sb[li] = t3

        def update_shadow(m, last=False):
            """After h_fm[:, m, :] residual update: refresh hr (+hsq/store)."""
            nc.vector.tensor_copy(hr[:, m, :], h_fm[:, m, :])
            if last:
                for cs in HALVES:
                    nc.sync.dma_start(hbT[m * 128 : (m + 1) * 128, cs], h_fm[:, m, cs])
            else:
                nc.scalar.activation(hsq[:, m, :], h_fm[:, m, :], AF.Square)

        def emit_ln():
            """x_hat = (h - mu) * rstd -> returns bf16 act tile [128, 8, TPC]."""
            s12 = p_s12.tile([33, TPC], F32, tag="s12")
            for cs in HALVES:
                for k in range(8):
                    nc.tensor.matmul(
                        s12[0:1, cs], ones128b[:], hr[:, k, cs],
                        start=(k == 0), stop=(k == 7),
                    )
                for k in range(8):
                    nc.tensor.matmul(
                        s12[32:33, cs], ones128b[:], hsq[:, k, cs],
                        start=(k == 0), stop=(k == 7),
                    )
            mu_t = p_st.tile([1, TPC], F32, tag="mu")
            var_t = p_st.tile([1, TPC], F32, tag="var")
            rstd_t = p_st.tile([1, TPC], F32R, tag="rstd")
            bt_t = p_st.tile([1, TPC], F32R, tag="bt")
            abA = p_bc.tile([128, TPC], F32, tag="bc", name="abA")
            bbB = p_bc.tile([128, TPC], F32, tag="bc", name="bbB")
            with nc.allow_low_precision(reason="LN rows feed f32r matmuls"):
                for cs in HALVES:
                    nc.vector.tensor_scalar_mul(mu_t[:, cs], s12[0:1, cs], 1.0 / DIM)
                    nc.vector.tensor_scalar_mul(var_t[:, cs], s12[32:33, cs], 1.0 / DIM)
                    # var = E[x^2] - mu^2  (musq = -mu*mu, then add)
                    musq = p_st.tile([1, TPC], F32, tag="musq")
                    nc.vector.scalar_tensor_tensor(
                        musq[:, cs], mu_t[:, cs], -1.0, mu_t[:, cs], OP.mult, OP.mult
                    )
                    nc.vector.tensor_add(var_t[:, cs], var_t[:, cs], musq[:, cs])
                    # var+eps > 0, so 1/sqrt(|x|) == rsqrt
                    nc.scalar.activation(
                        rstd_t[:, cs], var_t[:, cs], AF.Abs_reciprocal_sqrt,
                        bias=eps_t[:], scale=1.0,
                    )
                    nc.vector.scalar_tensor_tensor(
                        bt_t[:, cs], mu_t[:, cs], -1.0, rstd_t[:, cs],
                        OP.mult, OP.mult,
                    )
                    nc.tensor.matmul(
                        abA[:, cs], onesrow[:], rstd_t[:, cs], start=True, stop=True
                    )
                    nc.tensor.matmul(
                        bbB[:, cs], onesrow[:], bt_t[:, cs], start=True, stop=True
                    )
            # bf16 copies of the broadcast rows: the apply then runs all-16-bit
            # on the DVE (2x throughput) reading the bf16 shadow hr
            abA_b = p_st.tile([128, TPC], BF16, tag="abA_b")
            bbB_b = p_st.tile([128, TPC], BF16, tag="bbB_b")
            dst = p_act.tile([128, 8, TPC], BF16, tag="act")
            with nc.allow_low_precision(reason="bf16 matmul inputs"):
                for cs in HALVES:
                    nc.vector.tensor_copy(abA_b[:, cs], abA[:, cs])
                    nc.vector.tensor_copy(bbB_b[:, cs], bbB[:, cs])
                for k in range(8):
                    nc.vector.tensor_mul(dst[:, k, :], hr[:, k, :], abA_b[:])
                    nc.vector.tensor_add(dst[:, k, :], dst[:, k, :], bbB_b[:])
            return dst

        def matmul_stream(src_fm, wdram, kt, mt, epilogue):
            """dst[m] = sum_k w[k,m].T @ src[k], feature-major, full 512 moving.

            src_fm: [128, kt, TPC] bf16; wdram: [kt*128, mt*128] bf16.
            epilogue(m, acc) consumes the accumulated PSUM tile.
            """
            for mg in range((mt + 3) // 4):
                mls = [ml for ml in range(4) if mg * 4 + ml < mt]
                accs = {}
                for k in range(kt):
                    wt = p_w.tile([128, 512], BF16, tag="wt")
                    nc.sync.dma_start(
                        wt[:, : len(mls) * 128],
                        wdram[
                            k * 128 : (k + 1) * 128,
                            mg * 512 : mg * 512 + len(mls) * 128,
                        ],
                    )
                    for ml in mls:
                        if k == 0:
                            accs[ml] = p_mm.tile(
                                [128, TPC], F32, tag="mm", name=f"acc{ml}"
                            )
                        nc.tensor.matmul(
                            accs[ml][:],
                            wt[:, ml * 128 : (ml + 1) * 128],
                            src_fm[:, k, :],
                            start=(k == 0),
                            stop=(k == kt - 1),
                        )
                for ml in mls:
                    epilogue(mg * 4 + ml, accs[ml])

        for li in range(2):
            # --- attention (seq len 1): h += LN1(h) @ wv' @ wo + c_att ---
            a_fm = emit_ln()
            tmp_fm = p_tmp.tile([128, 8, TPC], BF16, tag="tmp")

            def ep_tmp(m, acc):
                nc.vector.tensor_copy(tmp_fm[:, m, :], acc[:])

            matmul_stream(a_fm, wvf[li], 8, 8, ep_tmp)

            def ep_resid_att(m, acc, li=li):
                nc.vector.scalar_tensor_tensor(
                    h_fm[:, m, :], acc[:], catt_sb[li][:, m : m + 1], h_fm[:, m, :],
                    OP.add, OP.add,
                )
                update_shadow(m)

            matmul_stream(tmp_fm, wof[li], 8, 8, ep_resid_att)

            # --- mlp: h += gelu(LN2(h) @ w1' + b1') @ w2 + b2 ---
            m_fm = emit_ln()
            g_fm = p_g.tile([128, 32, TPC], BF16, tag="g")

            def ep_gelu(m, acc, li=li):
                nc.scalar.activation(
                    g_fm[:, m, :],
                    acc[:],
                    AF.Gelu_apprx_tanh,
                    bias=b1_sb[li][:, m : m + 1],
                    scale=1.0,
                )

            matmul_stream(m_fm, w1f[li], 8, 32, ep_gelu)

            last = li == 1

            def ep_resid_mlp(m, acc, li=li, last=last):
                nc.vector.scalar_tensor_tensor(
                    h_fm[:, m, :], acc[:], b2_sb[li][:, m : m + 1], h_fm[:, m, :],
                    OP.add, OP.add,
                )
                update_shadow(m, last=last)

            matmul_stream(g_fm, w2f[li], 32, 8, ep_resid_mlp)

        # --- head: logits[v, t] = head_w[v, :] @ hb[:, t], full vocab ---
        for mg in range((NVG + 3) // 4):
            mls = [ml for ml in range(4) if mg * 4 + ml < NVG]
            w_ = len(mls) * 128
            wtbs = []
            for kc in range(2):
                wtb = p_wB.tile([128, 4, 512], BF16, tag="wtb")
                nc.sync.dma_start(
                    wtb[:, :, :w_],
                    hwTd[
                        kc * 512 : (kc + 1) * 512, mg * 512 : mg * 512 + w_
                    ].rearrange("(k p) v -> p k v", p=128),
                )
                wtbs.append(wtb)
            accs = {}
            for k in range(8):
                wtb = wtbs[k // 4]
                for ml in mls:
                    if k == 0:
                        accs[ml] = p_mm.tile([128, TPC], F32, tag="mm", name=f"ha{ml}")
                    nc.tensor.matmul(
                        accs[ml][:],
                        wtb[:, k % 4, ml * 128 : (ml + 1) * 128],
                        hr[:, k, :],
                        start=(k == 0),
                        stop=(k == 7),
                    )
            lo = p_lo.tile([128, 4, TPC], BF16, tag="lo")
            with nc.allow_low_precision(reason="bf16 logits output"):
                for ml in mls:
                    nc.vector.tensor_copy(lo[:, ml, :], accs[ml][:])
            if mg >= (NVG + 3) // 4 - 3:
                # near the end: per-vocab-tile stores spread the drain across
                # queues so the kernel doesn't tail-wait on one 512KB DMA
                for ml in mls:
                    vg = mg * 4 + ml
                    nc.sync.dma_start(
                        logT[vg * 128 : (vg + 1) * 128, :], lo[:, ml, :]
                    )
            else:
                nc.sync.dma_start(
                    logT[mg * 512 : mg * 512 + w_, :].rearrange(
                        "(g p) t -> p g t", p=128
                    ),
                    lo[:, : len(mls), :],
                )

    nc.compile()
    return nc


def _get():
    if "F" not in _cache:
        _cache["F"] = _build_F()
    return _cache["F"]


# --------------------------------------------------------------------------
# Host side
# --------------------------------------------------------------------------

def _gelu_tanh(x):
    return 0.5 * x * (1.0 + np.tanh(0.7978845608028654 * (x + 0.044715 * x * x * x)))


def _host_block1(hb, inputs):
    """Block-1 layers (li=2,3) + head, fp32 numpy, for non-exiting tokens."""
    hb = hb.astype(np.float32)
    for li in (2, 3):
        mu = hb.mean(-1, keepdims=True, dtype=np.float32)
        var = hb.var(-1, keepdims=True, dtype=np.float32)
        a = (hb - mu) / np.sqrt(var + LN_EPS)
        a = a * inputs["ln1_s"][li] + inputs["ln1_b"][li]
        hb = hb + (a @ inputs["wv"][li]) @ inputs["wo"][li]
        mu = hb.mean(-1, keepdims=True, dtype=np.float32)
        var = hb.var(-1, keepdims=True, dtype=np.float32)
        m = (hb - mu) / np.sqrt(var + LN_EPS)
        m = m * inputs["ln2_s"][li] + inputs["ln2_b"][li]
        hb = hb + _gelu_tanh(m @ inputs["w1"][li] + inputs["b1"][li]) @ inputs["w2"][
            li
        ] + inputs["b2"][li]
    return hb @ np.asarray(inputs["head_w"], np.float32).T


def kernel(**inputs):
    x = np.asarray(inputs["x"]).reshape(-1).astype(np.int64)
    emb = np.asarray(inputs["emb"], dtype=np.float32)
    head_w = np.asarray(inputs["head_w"], dtype=np.float32)
    f32c = lambda k: np.ascontiguousarray(np.asarray(inputs[k], dtype=np.float32))

    h0 = emb[x]  # [T, DIM]

    wv = f32c("wv")[:2]
    wo = f32c("wo")[:2]
    w1 = f32c("w1")[:2]
    w2 = f32c("w2")[:2]
    ln1s, ln1b = f32c("ln1_s")[:2], f32c("ln1_b")[:2]
    ln2s, ln2b = f32c("ln2_s")[:2], f32c("ln2_b")[:2]
    b1, b2 = f32c("b1")[:2], f32c("b2")[:2]

    bf = lambda a: np.ascontiguousarray(a).astype(ml_dtypes.bfloat16)
    wvf = bf(ln1s[:, :, None] * wv)                       # fold ln1 scale
    wof = bf(wo)
    w1f = bf(ln2s[:, :, None] * w1)                       # fold ln2 scale
    w2f = bf(w2)
    catt = np.einsum("ld,ldm->lm", ln1b, wv, optimize=True)
    catt = np.einsum("ld,ldm->lm", catt, wo, optimize=True).astype(np.float32)
    b1f = (b1 + np.einsum("ld,ldm->lm", ln2b, w1, optimize=True)).astype(np.float32)
    # pre-transposed per-partition bias layouts: [L, 128, M]
    tp = lambda a, m: np.ascontiguousarray(
        a.reshape(2, m, 128).transpose(0, 2, 1).astype(np.float32)
    )
    hwT = np.zeros((DIM, VP2), ml_dtypes.bfloat16)
    hwT[:, :VOCAB] = head_w.T.astype(ml_dtypes.bfloat16)

    ncF = _get()
    wF = {
        "wvf": wvf, "wof": wof, "w1f": w1f, "w2f": w2f,
        "b1ft": tp(b1f, DFF // 128), "b2t": tp(b2, DIM // 128),
        "cattt": tp(catt, DIM // 128), "hwT": hwT,
    }
    in_maps = []
    for c in range(NCORES):
        m = dict(wF)
        m["hT"] = np.ascontiguousarray(h0[c * TPC : (c + 1) * TPC].T)
        in_maps.append(m)
    res = run_bass_kernel_spmd(
        ncF, in_maps, core_ids=list(range(NCORES)), trace=TRACE
    )
    if TRACE:
        LAST_EXEC_NS["F"] = res.exec_time_ns
        LAST_PROFILE["F"] = res

    out = np.empty((T, VOCAB), np.float32)
    for c in range(NCORES):
        L = res.results[c]["logT"]  # [VP2, TPC] bf16
        out[c * TPC : (c + 1) * TPC, :] = L[:VOCAB].T.astype(np.float32)
    hbT = np.concatenate(
        [res.results[c]["hbT"] for c in range(NCORES)], axis=1
    )  # [DIM, T]

    # host softmax stats (chunked): max_prob = exp(M - MHAT) / sum exp(l - MHAT)
    M = np.empty(T, np.float32)
    Z = np.empty(T, np.float32)
    for i in range(0, T, 256):
        chunk = out[i : i + 256]
        M[i : i + 256] = chunk.max(1)
        Z[i : i + 256] = np.exp(chunk - MHAT, dtype=np.float32).sum(
            1, dtype=np.float32
        )
    max_prob = np.exp(M - MHAT).astype(np.float32) / Z
    cont = ~(max_prob >= THRESH)
    if cont.any():
        idx = np.where(cont)[0]
        out[idx] = _host_block1(hbT.T[idx], inputs)

    return out.reshape(tuple(np.asarray(inputs["x"]).shape) + (VOCAB,))


# revision 20
# speedup vs baseline: 1.4726x; 1.0083x over previous
"""LEGOTransformer (moe_routing early-exit) Trainium2 Bass kernel.

Reference semantics: tokens run through block0 (layers 0,1), compute
logits0 = hb0 @ head_w.T; tokens whose max softmax prob >= 1e-4 exit and
keep logits0. Remaining tokens run block1 (layers 2,3) from hb0 and take
logits1 (last block always writes active tokens).

Single fused launch, fully token-sharded (512 tok/core, no collectives):

  Layers: embedding rows -> 2 transformer layers, feature-major
    activations ([D, tok] in SBUF). LN scale/bias are folded into the
    weights host-side (wv' = s1*wv, w1' = s2*w1, c_att = (ln1_b@wv)@wo,
    b1' = b1 + ln2_b@w1), so the device LN only computes
    x_hat = (x-mu)*rstd via per-token rows A=rstd, B=-mu*rstd (broadcast
    across partitions with two K=1 matmuls). Stats come from bf16 shadow
    copies hr=h, hsq=h*h maintained in the matmul epilogues, so the
    stats matmuls run at full PE rate and are ready immediately.
    Stats/chain are split into token halves so the vector chain of one
    half hides under the other half's tensor work (keeps the PE p-state
    at max clock). Weights and matmul activations are bf16 (full PE
    rate, half the DMA); the residual stream h stays fp32.

  Head: each core computes logits for its OWN 512 tokens over the FULL
    vocab (padded to 50304 = 393*128), reusing the bf16 shadow hr as the
    moving operand: out[vocab128, tok] tiles, written bf16 to DRAM in
    vocab-major layout [50304, 512]; the host transposes/casts when
    assembling the [T, VOCAB] fp32 output. No cross-core collective and
    no second launch: the head stream starts as soon as the last layer's
    epilogues produce hr.

  Host: max-softmax exit mask computed from the full logits on host
    (identical decision to reference's max softmax >= 1e-4); tokens that
    do not exit (none for this input distribution, but handled honestly)
    get block1 + their logits row recomputed on host in fp32 numpy and
    patched in.
"""

import sys

sys.path.insert(0, "/opt/trn_rl_repo")

from contextlib import ExitStack

import ml_dtypes
import numpy as np

from concourse import bacc, tile, mybir
from concourse.bass_utils import run_bass_kernel_spmd

F32 = mybir.dt.float32
F32R = mybir.dt.float32r
BF16 = mybir.dt.bfloat16
AF = mybir.ActivationFunctionType
OP = mybir.AluOpType

VOCAB = 50257
DIM = 1024
DFF = 4096
T = 4096
NCORES = 8
TPC = T // NCORES          # tokens per core
NVG = 393                  # vocab 128-tiles (393*128 = 50304 >= 50257)
VP2 = NVG * 128
LN_EPS = 1e-5
MHAT = 16.0                # fixed exp shift for host softmax stats
THRESH = 1e-4
HALVES = (slice(0, TPC // 2), slice(TPC // 2, TPC))

_cache = {}

# test-harness knobs (harness never touches these; defaults are production)
TRACE = False
LAST_EXEC_NS = {}
LAST_PROFILE = {}


# --------------------------------------------------------------------------
# Fused launch: two transformer layers + full-vocab head, token-sharded
# --------------------------------------------------------------------------

def _build_F():
    nc = bacc.Bacc(None, target_bir_lowering=False)
    hT = nc.declare_dram_parameter("hT", [DIM, TPC], F32, isOutput=False)
    wvf = nc.declare_dram_parameter("wvf", [2, DIM, DIM], BF16, isOutput=False)
    wof = nc.declare_dram_parameter("wof", [2, DIM, DIM], BF16, isOutput=False)
    w1f = nc.declare_dram_parameter("w1f", [2, DIM, DFF], BF16, isOutput=False)
    w2f = nc.declare_dram_parameter("w2f", [2, DFF, DIM], BF16, isOutput=False)
    b1d = nc.declare_dram_parameter("b1ft", [2, 128, DFF // 128], F32, isOutput=False)
    b2d = nc.declare_dram_parameter("b2t", [2, 128, DIM // 128], F32, isOutput=False)
    cattd = nc.declare_dram_parameter("cattt", [2, 128, DIM // 128], F32, isOutput=False)
    hwTd = nc.declare_dram_parameter("hwT", [DIM, VP2], BF16, isOutput=False)
    hbT = nc.declare_dram_parameter("hbT", [DIM, TPC], F32, isOutput=True)
    logT = nc.declare_dram_parameter("logT", [VP2, TPC], BF16, isOutput=True)

    with tile.TileContext(nc) as tc, ExitStack() as ctx:
        p_h = ctx.enter_context(tc.tile_pool(name="p_h", bufs=1))
        p_hr = ctx.enter_context(tc.tile_pool(name="p_hr", bufs=1))
        p_act = ctx.enter_context(tc.tile_pool(name="p_act", bufs=2))
        p_tmp = ctx.enter_context(tc.tile_pool(name="p_tmp", bufs=1))
        p_g = ctx.enter_context(tc.tile_pool(name="p_g", bufs=1))
        p_w = ctx.enter_context(tc.tile_pool(name="p_w", bufs=10))
        p_wB = ctx.enter_context(tc.tile_pool(name="p_wB", bufs=6))
        p_lo = ctx.enter_context(tc.tile_pool(name="p_lo", bufs=3))
        p_st = ctx.enter_context(tc.tile_pool(name="p_st", bufs=2))
        p_c = ctx.enter_context(tc.tile_pool(name="p_c", bufs=1))
        p_mm = ctx.enter_context(tc.tile_pool(name="p_mm", bufs=5, space="PSUM"))
        p_bc = ctx.enter_context(tc.tile_pool(name="p_bc", bufs=2, space="PSUM"))
        p_s12 = ctx.enter_context(tc.tile_pool(name="p_s12", bufs=1, space="PSUM"))

        ones128f = p_c.tile([128, 1], F32, tag="ones128f")
        nc.gpsimd.memset(ones128f[:], 1.0)
        ones128b = p_c.tile([128, 1], BF16, tag="ones128b")
        nc.vector.tensor_copy(ones128b[:], ones128f[:])
        eps_t = p_c.tile([1, 1], F32, tag="eps")
        nc.gpsimd.memset(eps_t[:], LN_EPS)
        rowf = p_c.tile([1, 128], F32, tag="rowf")
        nc.gpsimd.memset(rowf[:], 1.0)
        onesrow = p_c.tile([1, 128], F32R, tag="onesrow")  # stationary for bcasts
        nc.vector.tensor_copy(onesrow[:], rowf[:])

        h_fm = p_h.tile([128, 8, TPC], F32, tag="h")
        hr = p_hr.tile([128, 8, TPC], BF16, tag="hr")
        hsq = p_hr.tile([128, 8, TPC], BF16, tag="hsq")
        # chunked load + immediate bf16 shadow prep (per k, per half);
        # issued before the bias loads so the first LN isn't queued behind them
        for k in range(8):
            for ci, cs in enumerate(HALVES):
                eng = nc.gpsimd if (k * 2 + ci) % 2 == 0 else nc.sync
                eng.dma_start(h_fm[:, k, cs], hT[k * 128 : (k + 1) * 128, cs])
                nc.vector.tensor_copy(hr[:, k, cs], h_fm[:, k, cs])
                nc.scalar.activation(hsq[:, k, cs], h_fm[:, k, cs], AF.Square)

        b1_sb = {}
        b2_sb = {}
        catt_sb = {}
        for li in range(2):
            t1 = p_c.tile([128, DFF // 128], F32, tag=f"b1_{li}")
            nc.sync.dma_start(t1[:], b1d[li])
            b1_sb[li] = t1
            t2 = p_c.tile([128, DIM // 128], F32, tag=f"b2_{li}")
            nc.sync.dma_start(t2[:], b2d[li])
            b2_sb[li] = t2
            t3 = p_c.tile([128, DIM // 128], F32, tag=f"catt_{li}")
            nc.sync.dma_start(t3[:], cattd[li])
            catt_sb[li] = t3

        def update_shadow(m, last=False):
            """After h_fm[:, m, :] residual update: refresh hr (+hsq/store)."""
            nc.vector.tensor_copy(hr[:, m, :], h_fm[:, m, :])
            if last:
                for cs in HALVES:
                    nc.sync.dma_start(hbT[m * 128 : (m + 1) * 128, cs], h_fm[:, m, cs])
            else:
                nc.scalar.activation(hsq[:, m, :], h_fm[:, m, :], AF.Square)

        def emit_ln():
            """x_hat = (h - mu) * rstd -> returns bf16 act tile [128, 8, TPC]."""
            s12 = p_s12.tile([33, TPC], F32, tag="s12")
            for cs in HALVES:
                for k in range(8):
                    nc.tensor.matmul(
                        s12[0:1, cs], ones128b[:], hr[:, k, cs],
                        start=(k == 0), stop=(k == 7),
                    )
                for k in range(8):
                    nc.tensor.matmul(
                        s12[32:33, cs], ones128b[:], hsq[:, k, cs],
                        start=(k == 0), stop=(k == 7),
                    )
            mu_t = p_st.tile([1, TPC], F32, tag="mu")
            var_t = p_st.tile([1, TPC], F32, tag="var")
            rstd_t = p_st.tile([1, TPC], F32R, tag="rstd")
            bt_t = p_st.tile([1, TPC], F32R, tag="bt")
            abA = p_bc.tile([128, TPC], F32, tag="bc", name="abA")
            bbB = p_bc.tile([128, TPC], F32, tag="bc", name="bbB")
            # bf16 copies of the broadcast rows: the apply then runs all-16-bit
            # on the DVE (2x throughput) reading the bf16 shadow hr
            abA_b = p_st.tile([128, TPC], BF16, tag="abA_b")
            bbB_b = p_st.tile([128, TPC], BF16, tag="bbB_b")
            with nc.allow_low_precision(reason="LN rows feed f32r matmuls"):
                for cs in HALVES:
                    nc.vector.tensor_scalar_mul(mu_t[:, cs], s12[0:1, cs], 1.0 / DIM)
                    # E[x^2] on the scalar engine, in parallel with mu
                    nc.scalar.activation(
                        var_t[:, cs], s12[32:33, cs], AF.Copy, scale=1.0 / DIM
                    )
                    # var = E[x^2] - mu^2  (musq = -mu*mu, then add)
                    musq = p_st.tile([1, TPC], F32, tag="musq")
                    nc.vector.scalar_tensor_tensor(
                        musq[:, cs], mu_t[:, cs], -1.0, mu_t[:, cs], OP.mult, OP.mult
                    )
                    nc.vector.tensor_add(var_t[:, cs], var_t[:, cs], musq[:, cs])
                    # var+eps > 0, so 1/sqrt(|x|) == rsqrt
                    nc.scalar.activation(
                        rstd_t[:, cs], var_t[:, cs], AF.Abs_reciprocal_sqrt,
                        bias=eps_t[:], scale=1.0,
                    )
                    nc.vector.scalar_tensor_tensor(
                        bt_t[:, cs], mu_t[:, cs], -1.0, rstd_t[:, cs],
                        OP.mult, OP.mult,
                    )
                    nc.tensor.matmul(
                        abA[:, cs], onesrow[:], rstd_t[:, cs], start=True, stop=True
                    )
                    nc.tensor.matmul(
                        bbB[:, cs], onesrow[:], bt_t[:, cs], start=True, stop=True
                    )
                    # casts split across engines so they overlap
                    nc.scalar.activation(abA_b[:, cs], abA[:, cs], AF.Copy)
                    nc.vector.tensor_copy(bbB_b[:, cs], bbB[:, cs])
            dst = p_act.tile([128, 8, TPC], BF16, tag="act")
            with nc.allow_low_precision(reason="bf16 matmul inputs"):
                for k in range(8):
                    nc.vector.tensor_mul(dst[:, k, :], hr[:, k, :], abA_b[:])
                    nc.vector.tensor_add(dst[:, k, :], dst[:, k, :], bbB_b[:])
            return dst

        def matmul_stream(src_fm, wdram, kt, mt, epilogue):
            """dst[m] = sum_k w[k,m].T @ src[k], feature-major, full 512 moving.

            src_fm: [128, kt, TPC] bf16; wdram: [kt*128, mt*128] bf16.
            epilogue(m, acc) consumes the accumulated PSUM tile.
            """
            for mg in range((mt + 3) // 4):
                mls = [ml for ml in range(4) if mg * 4 + ml < mt]
                accs = {}
                for k in range(kt):
                    wt = p_w.tile([128, 512], BF16, tag="wt")
                    nc.sync.dma_start(
                        wt[:, : len(mls) * 128],
                        wdram[
                            k * 128 : (k + 1) * 128,
                            mg * 512 : mg * 512 + len(mls) * 128,
                        ],
                    )
                    for ml in mls:
                        if k == 0:
                            accs[ml] = p_mm.tile(
                                [128, TPC], F32, tag="mm", name=f"acc{ml}"
                            )
                        nc.tensor.matmul(
                            accs[ml][:],
                            wt[:, ml * 128 : (ml + 1) * 128],
                            src_fm[:, k, :],
                            start=(k == 0),
                            stop=(k == kt - 1),
                        )
                for ml in mls:
                    epilogue(mg * 4 + ml, accs[ml])

        for li in range(2):
            # --- attention (seq len 1): h += LN1(h) @ wv' @ wo + c_att ---
            a_fm = emit_ln()
            tmp_fm = p_tmp.tile([128, 8, TPC], BF16, tag="tmp")

            def ep_tmp(m, acc):
                nc.vector.tensor_copy(tmp_fm[:, m, :], acc[:])

            matmul_stream(a_fm, wvf[li], 8, 8, ep_tmp)

            def ep_resid_att(m, acc, li=li):
                nc.vector.scalar_tensor_tensor(
                    h_fm[:, m, :], acc[:], catt_sb[li][:, m : m + 1], h_fm[:, m, :],
                    OP.add, OP.add,
                )
                update_shadow(m)

            matmul_stream(tmp_fm, wof[li], 8, 8, ep_resid_att)

            # --- mlp: h += gelu(LN2(h) @ w1' + b1') @ w2 + b2 ---
            m_fm = emit_ln()
            g_fm = p_g.tile([128, 32, TPC], BF16, tag="g")

            def ep_gelu(m, acc, li=li):
                nc.scalar.activation(
                    g_fm[:, m, :],
                    acc[:],
                    AF.Gelu_apprx_tanh,
                    bias=b1_sb[li][:, m : m + 1],
                    scale=1.0,
                )

            matmul_stream(m_fm, w1f[li], 8, 32, ep_gelu)

            last = li == 1

            def ep_resid_mlp(m, acc, li=li, last=last):
                nc.vector.scalar_tensor_tensor(
                    h_fm[:, m, :], acc[:], b2_sb[li][:, m : m + 1], h_fm[:, m, :],
                    OP.add, OP.add,
                )
                update_shadow(m, last=last)

            matmul_stream(g_fm, w2f[li], 32, 8, ep_resid_mlp)

        # --- head: logits[v, t] = head_w[v, :] @ hb[:, t], full vocab ---
        for mg in range((NVG + 3) // 4):
            mls = [ml for ml in range(4) if mg * 4 + ml < NVG]
            w_ = len(mls) * 128
            wtbs = []
            for kc in range(2):
                wtb = p_wB.tile([128, 4, 512], BF16, tag="wtb")
                nc.sync.dma_start(
                    wtb[:, :, :w_],
                    hwTd[
                        kc * 512 : (kc + 1) * 512, mg * 512 : mg * 512 + w_
                    ].rearrange("(k p) v -> p k v", p=128),
                )
                wtbs.append(wtb)
            accs = {}
            for k in range(8):
                wtb = wtbs[k // 4]
                for ml in mls:
                    if k == 0:
                        accs[ml] = p_mm.tile([128, TPC], F32, tag="mm", name=f"ha{ml}")
                    nc.tensor.matmul(
                        accs[ml][:],
                        wtb[:, k % 4, ml * 128 : (ml + 1) * 128],
                        hr[:, k, :],
                        start=(k == 0),
                        stop=(k == 7),
                    )
            lo = p_lo.tile([128, 4, TPC], BF16, tag="lo")
            with nc.allow_low_precision(reason="bf16 logits output"):
                for ml in mls:
                    nc.vector.tensor_copy(lo[:, ml, :], accs[ml][:])
            if mg >= (NVG + 3) // 4 - 3:
                # near the end: per-vocab-tile stores spread the drain across
                # queues so the kernel doesn't tail-wait on one 512KB DMA
                for ml in mls:
                    vg = mg * 4 + ml
                    nc.sync.dma_start(
                        logT[vg * 128 : (vg + 1) * 128, :], lo[:, ml, :]
                    )
            else:
                nc.sync.dma_start(
                    logT[mg * 512 : mg * 512 + w_, :].rearrange(
                        "(g p) t -> p g t", p=128
                    ),
                    lo[:, : len(mls), :],
                )

    nc.compile()
    return nc


def _get():
    if "F" not in _cache:
        _cache["F"] = _build_F()
    return _cache["F"]


# --------------------------------------------------------------------------
# Host side
# --------------------------------------------------------------------------

def _gelu_tanh(x):
    return 0.5 * x * (1.0 + np.tanh(0.7978845608028654 * (x + 0.044715 * x * x * x)))


def _host_block1(hb, inputs):
    """Block-1 layers (li=2,3) + head, fp32 numpy, for non-exiting tokens."""
    hb = hb.astype(np.float32)
    for li in (2, 3):
        mu = hb.mean(-1, keepdims=True, dtype=np.float32)
        var = hb.var(-1, keepdims=True, dtype=np.float32)
        a = (hb - mu) / np.sqrt(var + LN_EPS)
        a = a * inputs["ln1_s"][li] + inputs["ln1_b"][li]
        hb = hb + (a @ inputs["wv"][li]) @ inputs["wo"][li]
        mu = hb.mean(-1, keepdims=True, dtype=np.float32)
        var = hb.var(-1, keepdims=True, dtype=np.float32)
        m = (hb - mu) / np.sqrt(var + LN_EPS)
        m = m * inputs["ln2_s"][li] + inputs["ln2_b"][li]
        hb = hb + _gelu_tanh(m @ inputs["w1"][li] + inputs["b1"][li]) @ inputs["w2"][
            li
        ] + inputs["b2"][li]
    return hb @ np.asarray(inputs["head_w"], np.float32).T


def kernel(**inputs):
    x = np.asarray(inputs["x"]).reshape(-1).astype(np.int64)
    emb = np.asarray(inputs["emb"], dtype=np.float32)
    head_w = np.asarray(inputs["head_w"], dtype=np.float32)
    f32c = lambda k: np.ascontiguousarray(np.asarray(inputs[k], dtype=np.float32))

    h0 = emb[x]  # [T, DIM]

    wv = f32c("wv")[:2]
    wo = f32c("wo")[:2]
    w1 = f32c("w1")[:2]
    w2 = f32c("w2")[:2]
    ln1s, ln1b = f32c("ln1_s")[:2], f32c("ln1_b")[:2]
    ln2s, ln2b = f32c("ln2_s")[:2], f32c("ln2_b")[:2]
    b1, b2 = f32c("b1")[:2], f32c("b2")[:2]

    bf = lambda a: np.ascontiguousarray(a).astype(ml_dtypes.bfloat16)
    wvf = bf(ln1s[:, :, None] * wv)                       # fold ln1 scale
    wof = bf(wo)
    w1f = bf(ln2s[:, :, None] * w1)                       # fold ln2 scale
    w2f = bf(w2)
    catt = np.einsum("ld,ldm->lm", ln1b, wv, optimize=True)
    catt = np.einsum("ld,ldm->lm", catt, wo, optimize=True).astype(np.float32)
    b1f = (b1 + np.einsum("ld,ldm->lm", ln2b, w1, optimize=True)).astype(np.float32)
    # pre-transposed per-partition bias layouts: [L, 128, M]
    tp = lambda a, m: np.ascontiguousarray(
        a.reshape(2, m, 128).transpose(0, 2, 1).astype(np.float32)
    )
    hwT = np.zeros((DIM, VP2), ml_dtypes.bfloat16)
    hwT[:, :VOCAB] = head_w.T.astype(ml_dtypes.bfloat16)

    ncF = _get()
    wF = {
        "wvf": wvf, "wof": wof, "w1f": w1f, "w2f": w2f,
        "b1ft": tp(b1f, DFF // 128), "b2t": tp(b2, DIM // 128),
        "cattt": tp(catt, DIM // 128), "hwT": hwT,
    }
    in_maps = []
    for c in range(NCORES):
        m = dict(wF)
        m["hT"] = np.ascontiguousarray(h0[c * TPC : (c + 1) * TPC].T)
        in_maps.append(m)
    res = run_bass_kernel_spmd(
        ncF, in_maps, core_ids=list(range(NCORES)), trace=TRACE
    )
    if TRACE:
        LAST_EXEC_NS["F"] = res.exec_time_ns
        LAST_PROFILE["F"] = res

    out = np.empty((T, VOCAB), np.float32)
    for c in range(NCORES):
        L = res.results[c]["logT"]  # [VP2, TPC] bf16
        out[c * TPC : (c + 1) * TPC, :] = L[:VOCAB].T.astype(np.float32)
    hbT = np.concatenate(
        [res.results[c]["hbT"] for c in range(NCORES)], axis=1
    )  # [DIM, T]

    # host softmax stats (chunked): max_prob = exp(M - MHAT) / sum exp(l - MHAT)
    M = np.empty(T, np.float32)
    Z = np.empty(T, np.float32)
    for i in range(0, T, 256):
        chunk = out[i : i + 256]
        M[i : i + 256] = chunk.max(1)
        Z[i : i + 256] = np.exp(chunk - MHAT, dtype=np.float32).sum(
            1, dtype=np.float32
        )
    max_prob = np.exp(M - MHAT).astype(np.float32) / Z
    cont = ~(max_prob >= THRESH)
    if cont.any():
        idx = np.where(cont)[0]
        out[idx] = _host_block1(hbT.T[idx], inputs)

    return out.reshape(tuple(np.asarray(inputs["x"]).shape) + (VOCAB,))
